# revision 1
# baseline (speedup 1.0000x reference)
"""Single-head attention (shared QKV weight) on 8 Trainium2 NeuronCores.

Problem: B=4, S=2048, D=E=1024
  Q = xq@Wq.T + bq ; K = xk@Wq.T + bq ; V = xv@Wq.T + bq
  out = softmax(mask(Q@K.T/sqrt(E))) @ V

Sharding: data-parallel over batch x query-halves -> 8 cores. Core c
handles batch b=c//2 and a causally-balanced set of 8 query tiles (128
rows each) so every core executes the same instruction stream with the
same FLOP count. Each core computes the full K/V projection of its
batch (replicated within the batch pair), its own Q projection, and
attention for its query tiles.

Math shortcuts (exact):
- K-bias adds a per-query constant to every score row -> cancels in
  softmax -> skipped.
- Q-bias is fused into the Q-projection PSUM eviction (per-partition
  bias in the e-major layout).
- V-bias: rows of softmax sum to 1, so out = P@Vraw/rowsum + bq; added
  once to the output tile.
- Scores are bounded (|s|/32 <~ 12 for unit-normal inputs), so softmax
  skips the max-subtraction; exp never overflows fp32 and the
  normalizer is applied to the PV output via a per-partition scale.

All matmuls run in float32r (4x the fp32 throughput, ~1.5e-4 rel err).
"""

import re

import numpy as np

import concourse.bass as bass
import concourse.mybir as mybir
import concourse.tile as tile
from concourse.masks import make_identity
from concourse.vector_clock import ScopedClock

F32 = mybir.dt.float32
F32R = mybir.dt.float32r
AF = mybir.ActivationFunctionType

B, S, D, E = 4, 2048, 1024, 1024
NCORES = 8
SCALE = 1.0 / 32.0  # E ** -0.5
NEG = -1.0e30

# Causally balanced q-tile assignment: global tile t (128 rows) needs
# keys up to kend = 512*ceil((t+1)/4). Halves get the same multiset of
# kend classes so the SPMD program is identical across cores.
TILES_H0 = [0, 1, 4, 5, 8, 9, 12, 13]
TILES_H1 = [2, 3, 6, 7, 10, 11, 14, 15]

# ---------------------------------------------------------------------------
# Workarounds for this container's walrus build, which rejects any
# instruction carrying more than one semaphore wait.
# ---------------------------------------------------------------------------

_split_counter = [0]


def _legalize_waits(nc):
    """Move all-but-one sem wait from each instruction onto single-wait
    NoOps inserted immediately before it on the same engine. Engines
    dispatch in order, so the nops' waits are satisfied before the
    instruction issues."""
    for f in nc.m.functions:
        for bb in f.blocks:
            insts = list(bb.instructions)
            out = []
            changed = False
            for inst in insts:
                si = inst.sync_info
                if si is not None and si.on_wait is not None and len(si.on_wait) > 1:
                    waits = list(si.on_wait)
                    for w in waits[:-1]:
                        _split_counter[0] += 1
                        nop = mybir.InstNoOp(
                            name=f"I-waitsplit-{_split_counter[0]}",
                            opcode="NoOp",
                            engine=inst.engine,
                            sync_info=mybir.SyncInfo(on_wait=[w], on_update=[]),
                        )
                        nc.register_instruction(nop)
                        out.append(nop)
                    si.on_wait = [waits[-1]]
                    changed = True
                out.append(inst)
            if changed:
                bb.instructions = out


class _TileContext(tile.TileContext):
    def __init__(self, nc, **kw):
        kw.setdefault("pool_alloc_mode", "queue")
        super().__init__(nc, **kw)

    def _drain_and_barrier(self, tick_clock, wait_clock):
        gc = tick_clock.global_clock
        m = re.search(r"\[([0-9, ]*)\]", repr(gc))
        ticks = (
            [int(x) for x in m.group(1).split(",")]
            if m and m.group(1).strip()
            else []
        )
        for p, t in [(i, t) for i, t in enumerate(ticks) if t > 0]:
            nop = self.nc.sync.nop(nofuse=True, hint="drain_split")
            sc = ScopedClock({})
            sc.require_at_least(None, p, t)
            wait_clock.add_sem_waits(nop.ins, sc)
        self.nc.sync.drain()
        self.nc.all_engine_barrier()
        assert self.sems is not None
        popped = self.nc._tile_sem_poison_stack.pop()
        assert popped is self._sem_poison
        self.nc.clear_and_free_semaphores(list(self.sems.allocated().values()))
        self.nc.all_engine_barrier()

    def __exit__(self, *args):
        r = super().__exit__(*args)
        _legalize_waits(self.nc)
        return r


# ---------------------------------------------------------------------------
# Device program (identical on all 8 cores).
# ---------------------------------------------------------------------------


def build_program(chunk_counts, mask_chunks, repeat=1):
    """chunk_counts: per q-tile number of 512-wide key chunks to process.
    mask_chunks: set of (q_tile_idx, chunk_idx) that get an additive mask
    tile (ordered mask DRAM array follows this order). repeat: run the
    whole body N times (timing aid; output identical)."""
    nmask = len(mask_chunks)
    mask_order = {qc: i for i, qc in enumerate(sorted(mask_chunks))}

    nc = bass.Bass("TRN2", target_bir_lowering=False, debug=False)
    wqT = nc.declare_dram_parameter("wqT", [D, E], F32R, isOutput=False)
    xqT = nc.declare_dram_parameter("xqT", [D, 1024], F32R, isOutput=False)
    xkT = nc.declare_dram_parameter("xkT", [D, S], F32R, isOutput=False)
    xvT = nc.declare_dram_parameter("xvT", [D, S], F32R, isOutput=False)
    bq8 = nc.declare_dram_parameter("bq8", [128, 8], F32, isOutput=False)
    bqb = nc.declare_dram_parameter("bqb", [128, E], F32, isOutput=False)
    if nmask:
        maskd = nc.declare_dram_parameter(
            "maskd", [nmask, 128, 512], F32, isOutput=False
        )
    out = nc.declare_dram_parameter("out", [1024, E], F32, isOutput=True)

    with _TileContext(nc) as tc:
        with (
            tc.tile_pool(name="const", bufs=1) as cpool,
            tc.tile_pool(name="big", bufs=1) as bpool,
        ):
            for _rep in range(repeat):
                wq_ctx = tc.tile_pool(name=f"wqpool{_rep}", bufs=1)
                wqpool = wq_ctx.__enter__()
                wq_sb = wqpool.tile([128, 8, E], F32R, tag="wq")
                nc.sync.dma_start(wq_sb[:], wqT.ap().rearrange("(t p) e -> p t e", p=128))
                bq8_sb = cpool.tile([128, 8], F32, tag="bq8")
                nc.sync.dma_start(bq8_sb[:], bq8[:])
                bqb_sb = cpool.tile([128, E], F32, tag="bqb")
                nc.sync.dma_start(bqb_sb[:], bqb[:])
                ident = cpool.tile([128, 128], F32, tag="ident")
                make_identity(nc, ident[:])

                q_sb = bpool.tile([128, 8, 1024], F32R, tag="q")
                k_sb = bpool.tile([128, 8, S], F32R, tag="k")
                v_sb = bpool.tile([128, 16, E], F32R, tag="v")

                # ---- projections ----
                with (
                    tc.tile_pool(name=f"pstage{_rep}", bufs=5) as stpool,
                    tc.tile_pool(name=f"projps{_rep}", bufs=8, space="PSUM") as ppsum,
                ):
                    # Q^T and K^T (e-major): out[e, s] += WqT[d, e].T @ xT[d, s]
                    for xT, dst, nch, with_bias in (
                        (xqT, q_sb, 2, True),
                        (xkT, k_sb, 4, False),
                    ):
                        for ch in range(nch):
                            pss = [
                                ppsum.tile([128, 512], F32, tag="pp", name=f"pp{i}")
                                for i in range(8)
                            ]
                            for dt in range(8):
                                xst = stpool.tile([128, 512], F32R, tag="xst")
                                nc.sync.dma_start(
                                    xst[:],
                                    xT[
                                        dt * 128 : (dt + 1) * 128,
                                        ch * 512 : (ch + 1) * 512,
                                    ],
                                )
                                for et in range(8):
                                    nc.tensor.matmul(
                                        pss[et][:],
                                        wq_sb[:, dt, et * 128 : (et + 1) * 128],
                                        xst[:],
                                        start=(dt == 0),
                                        stop=(dt == 7),
                                    )
                            for et in range(8):
                                if with_bias:
                                    nc.scalar.activation(
                                        dst[:, et, ch * 512 : (ch + 1) * 512],
                                        pss[et][:],
                                        AF.Identity,
                                        bias=bq8_sb[:, et : et + 1],
                                    )
                                else:
                                    nc.scalar.activation(
                                        dst[:, et, ch * 512 : (ch + 1) * 512],
                                        pss[et][:],
                                        AF.Copy,
                                    )

                    # V (s-major): out[s, e] += xvT[d, s].T @ WqT[d, e].
                    # 4 s-tiles per block -> 8 live PSUM groups, staged via
                    # the same deep [128, 512] pipeline as Q/K.
                    for sb4 in range(4):
                        pss = [
                            ppsum.tile([128, 512], F32, tag="pp", name=f"vp{i}")
                            for i in range(8)
                        ]
                        for dt in range(8):
                            xst = stpool.tile([128, 512], F32R, tag="xst")
                            nc.sync.dma_start(
                                xst[:],
                                xvT[
                                    dt * 128 : (dt + 1) * 128,
                                    sb4 * 512 : (sb4 + 1) * 512,
                                ],
                            )
                            for si in range(4):
                                for ec in range(2):
                                    nc.tensor.matmul(
                                        pss[si * 2 + ec][:],
                                        xst[:, si * 128 : (si + 1) * 128],
                                        wq_sb[:, dt, ec * 512 : (ec + 1) * 512],
                                        start=(dt == 0),
                                        stop=(dt == 7),
                                    )
                        for si in range(4):
                            for ec in range(2):
                                nc.vector.tensor_copy(
                                    v_sb[:, sb4 * 4 + si, ec * 512 : (ec + 1) * 512],
                                    pss[si * 2 + ec][:],
                                )

                # ---- attention ----
                wq_ctx.__exit__(None, None, None)
                with (
                    tc.tile_pool(name=f"work{_rep}", bufs=3) as wpool,
                    tc.tile_pool(name=f"small{_rep}", bufs=4) as spool,
                    tc.tile_pool(name=f"mstage{_rep}", bufs=2) as mpool,
                    tc.tile_pool(name=f"opool{_rep}", bufs=2) as opool,
                    tc.tile_pool(name=f"sps{_rep}", bufs=2, space="PSUM") as spsum,
                    tc.tile_pool(name=f"trps{_rep}", bufs=2, space="PSUM") as trpsum,
                    tc.tile_pool(name=f"ops{_rep}", bufs=2, space="PSUM") as opsum,
                ):
                    for qt in range(8):
                        ncha = chunk_counts[qt]
                        o_ps = opsum.tile([128, 1024], F32, tag="o")
                        rs = spool.tile([128, 1], F32, tag="rs")
                        for kc in range(ncha):
                            s_ps = spsum.tile([128, 512], F32, tag="s")
                            for et in range(8):
                                nc.tensor.matmul(
                                    s_ps[:],
                                    q_sb[:, et, qt * 128 : (qt + 1) * 128],
                                    k_sb[:, et, kc * 512 : (kc + 1) * 512],
                                    start=(et == 0),
                                    stop=(et == 7),
                                )
                            if (qt, kc) in mask_order:
                                msk = mpool.tile([128, 512], F32, tag="msk")
                                nc.sync.dma_start(msk[:], maskd[mask_order[(qt, kc)]])
                                nc.vector.tensor_add(s_ps[:], s_ps[:], msk[:])
                            p_sb = wpool.tile([128, 512], F32, tag="p")
                            part = spool.tile([128, 1], F32, tag="part")
                            nc.scalar.activation(
                                p_sb[:],
                                s_ps[:],
                                AF.Exp,
                                scale=SCALE,
                                accum_out=part[:],
                            )
                            if kc == 0:
                                nc.vector.tensor_copy(rs[:], part[:])
                            else:
                                nc.vector.tensor_add(rs[:], rs[:], part[:])
                            pT = wpool.tile([128, 512], F32R, tag="pt")
                            for j in range(4):
                                tr_ps = trpsum.tile([128, 128], F32, tag="tr")
                                nc.tensor.transpose(
                                    tr_ps[:], p_sb[:, j * 128 : (j + 1) * 128], ident[:]
                                )
                                nc.vector.tensor_copy(
                                    pT[:, j * 128 : (j + 1) * 128], tr_ps[:]
                                )
                            for j in range(4):
                                kidx = kc * 4 + j
                                for ec in range(2):
                                    nc.tensor.matmul(
                                        o_ps[:, ec * 512 : (ec + 1) * 512],
                                        pT[:, j * 128 : (j + 1) * 128],
                                        v_sb[:, kidx, ec * 512 : (ec + 1) * 512],
                                        start=(kidx == 0),
                                        stop=(kidx == ncha * 4 - 1),
                                    )
                        rcp = spool.tile([128, 1], F32, tag="rcp")
                        nc.vector.reciprocal(rcp[:], rs[:])
                        o_sb = opool.tile([128, E], F32, tag="osb")
                        nc.scalar.activation(o_sb[:], o_ps[:], AF.Copy, scale=rcp[:])
                        nc.vector.tensor_add(o_sb[:], o_sb[:], bqb_sb[:])
                        nc.sync.dma_start(out[qt * 128 : (qt + 1) * 128, :], o_sb[:])

    return nc


# ---------------------------------------------------------------------------
# Host wrapper.
# ---------------------------------------------------------------------------

_prog_cache = {}


def _get_program(variant, chunk_counts, mask_chunks):
    key = (variant, tuple(chunk_counts), tuple(sorted(mask_chunks)))
    if key not in _prog_cache:
        _prog_cache[key] = build_program(chunk_counts, mask_chunks)
    return _prog_cache[key]


def _analyze_mask(att_mask):
    """Return (chunk_counts per local tile slot, mask_chunks, tiles maps)."""
    causal = np.array_equal(
        att_mask, np.triu(np.ones((S, S), dtype=att_mask.dtype), 1)
    )
    if causal:
        # local slot i covers global tile TILES_H*[i]; kend class per slot
        chunk_counts = [1, 1, 2, 2, 3, 3, 4, 4]
        mask_chunks = {(qt, chunk_counts[qt] - 1) for qt in range(8)}
        return "causal", chunk_counts, mask_chunks
    if not att_mask.any():
        return "nomask", [4] * 8, set()
    return "generic", [4] * 8, {(qt, kc) for qt in range(8) for kc in range(4)}


def kernel(xq, xk, xv, Wq, bq, att_mask):
    from concourse.bass_utils import run_bass_kernel_spmd

    variant, chunk_counts, mask_chunks = _analyze_mask(np.asarray(att_mask))
    nc = _get_program(variant, chunk_counts, mask_chunks)

    xq = np.asarray(xq, dtype=np.float32)
    xk = np.asarray(xk, dtype=np.float32)
    xv = np.asarray(xv, dtype=np.float32)
    Wq = np.asarray(Wq, dtype=np.float32)
    bq = np.asarray(bq, dtype=np.float32)

    wqT = np.ascontiguousarray(Wq.T)  # [d, e]
    bq8 = np.ascontiguousarray(bq.reshape(8, 128).T)  # [128, 8]
    bqb = np.ascontiguousarray(np.broadcast_to(bq, (128, E)))

    mask_list = sorted(mask_chunks)
    tiles_by_half = (TILES_H0, TILES_H1)

    in_maps = []
    for c in range(NCORES):
        b, h = divmod(c, 2)
        tiles = tiles_by_half[h]
        rows = np.concatenate(
            [np.arange(t * 128, (t + 1) * 128) for t in tiles]
        )
        m = {
            "wqT": wqT,
            "xqT": np.ascontiguousarray(xq[b].T[:, rows]),
            "xkT": np.ascontiguousarray(xk[b].T),
            "xvT": np.ascontiguousarray(xv[b].T),
            "bq8": bq8,
            "bqb": bqb,
        }
        if mask_list:
            md = np.empty((len(mask_list), 128, 512), dtype=np.float32)
            for i, (qt, kc) in enumerate(mask_list):
                t = tiles[qt]
                md[i] = att_mask[
                    t * 128 : (t + 1) * 128, kc * 512 : (kc + 1) * 512
                ].astype(np.float32) * NEG
            m["maskd"] = md
        in_maps.append(m)

    res = run_bass_kernel_spmd(nc, in_maps, list(range(NCORES)))

    out = np.empty((B, S, E), dtype=np.float32)
    for c in range(NCORES):
        b, h = divmod(c, 2)
        tiles = tiles_by_half[h]
        oc = res.results[c]["out"]
        for i, t in enumerate(tiles):
            out[b, t * 128 : (t + 1) * 128, :] = oc[i * 128 : (i + 1) * 128, :]
    return out



# revision 41
# speedup vs baseline: 3.0070x; 3.0070x over previous
"""Single-head attention (shared QKV weight) on 8 Trainium2 NeuronCores.

Problem: B=4, S=2048, D=E=1024
  Q = xq@Wq.T + bq ; K = xk@Wq.T + bq ; V = xv@Wq.T + bq
  out = softmax(mask(Q@K.T/sqrt(E))) @ V

Sharding: data-parallel over batch x query-halves -> 8 cores. Core c
handles batch b=c//2 and half h=c%2 of the query tiles: even global
128-row tiles for h=0, odd for h=1, so slot i on every core processes
exactly i+1 key-chunks of 256 under the causal mask (identical SPMD
instruction stream, balanced FLOPs). Each core computes the full K/V
projection of its batch and its own Q projection.

All matmuls run in fp8e4 (e4m3) with DoubleRow perf mode: two 128-deep
contraction tiles per instruction at 0.5 PE cycles/row -> 4x fp32r
throughput. Scaling keeps everything in fp8 range:
- host supplies x (unit-normal, as-is) and 32*Wq in fp8
- projections evict PSUM/4 -> fp8 (values = 8x true Q/K/V)
- scores PSUM = 64 * QK^T_true; exp applies 1/(64*32) = 1/2048
- attention is k-major: scores land as S^T[k,q], exp writes fp8
  P^T[k,2,q] tiles directly usable as the PV stationary operand (no
  transposes); row-sums come from an N=1 matmul with an 8.0-vector
  (so rs = 8*rowsum matches o_ps = 8*P@V); the final
  out = o_ps*(1/rs) + bq is one scalar_tensor_tensor op.
- the causal boundary mask is applied ON the PE: an extra matmul per
  masked k-tile accumulates 240 * mask8[k,q] (mask8 in {0,-240}) =
  -57600 into the raw-score PSUM (|s| < 5e3), so exp -> 0 with no
  cross-engine hop before the exp.

Only Act and DVE can read PSUM (GPSIMD cannot), so each projection
PSUM group (one [128,2048] tile, 4 banks) is evicted by both lanes in
parallel (two [128,1024] halves), keeping per-group eviction wall time
under the group's matmul time.

Math shortcuts (exact): K-bias cancels in softmax; Q-bias fused into
the Q eviction; V-bias added via the fused final op (softmax rows sum
to 1); scores bounded (|s|/32 <~ 2) so softmax skips max-subtraction.
"""

import re
from collections import deque

import numpy as np
import ml_dtypes

import concourse.bass as bass
import concourse.mybir as mybir
import concourse.tile as tile
from concourse.vector_clock import ScopedClock

F32 = mybir.dt.float32
FP8 = mybir.dt.float8e4
BF16 = mybir.dt.bfloat16
NP_FP8 = ml_dtypes.float8_e4m3
NP_BF16 = ml_dtypes.bfloat16
AF = mybir.ActivationFunctionType
ALU = mybir.AluOpType
DR = mybir.MatmulPerfMode.DoubleRow

B, S, D, E = 4, 2048, 1024, 1024
NCORES = 8
W_SCALE = 32.0  # host multiplies Wq by this before fp8 conversion
QKV_SCALE = 8.0  # fp8 Q/K/V values are 8x the true values
EVICT_SCALE = QKV_SCALE / W_SCALE  # PSUM -> fp8 eviction multiplier
EXP_SCALE = (1.0 / 32.0) / (QKV_SCALE * QKV_SCALE)  # softmax scale / 64
MASK_VAL = 240.0  # ident240 @ mask8(-240) adds -57600 to raw scores

# Half h owns global q-tiles h, h+2, ..., h+14. Slot i (its i-th local
# 128-row tile) is global tile 2i+h and needs ceil((2i+h+1)/2) = i+1
# chunks of 256 keys -> identical instruction stream on all cores.
TILES_H0 = [0, 2, 4, 6, 8, 10, 12, 14]
TILES_H1 = [1, 3, 5, 7, 9, 11, 13, 15]

# ---------------------------------------------------------------------------
# Workarounds for this container's walrus build, which rejects any
# instruction carrying more than one semaphore wait.
# ---------------------------------------------------------------------------

_split_counter = [0]


def _legalize_waits(nc):
    """Move all-but-one sem wait from each instruction onto single-wait
    NoOps inserted immediately before it on the same engine. Engines
    dispatch in order, so the nops' waits are satisfied before the
    instruction issues."""
    for f in nc.m.functions:
        for bb in f.blocks:
            insts = list(bb.instructions)
            out = []
            changed = False
            for inst in insts:
                si = inst.sync_info
                if si is not None and si.on_wait is not None and len(si.on_wait) > 1:
                    waits = list(si.on_wait)
                    for w in waits[:-1]:
                        _split_counter[0] += 1
                        nop = mybir.InstNoOp(
                            name=f"I-waitsplit-{_split_counter[0]}",
                            opcode="NoOp",
                            engine=inst.engine,
                            sync_info=mybir.SyncInfo(on_wait=[w], on_update=[]),
                        )
                        nc.register_instruction(nop)
                        out.append(nop)
                    si.on_wait = [waits[-1]]
                    changed = True
                out.append(inst)
            if changed:
                bb.instructions = out


class _TileContext(tile.TileContext):
    def __init__(self, nc, **kw):
        kw.setdefault("pool_alloc_mode", "queue")
        super().__init__(nc, **kw)

    def _drain_and_barrier(self, tick_clock, wait_clock):
        gc = tick_clock.global_clock
        m = re.search(r"\[([0-9, ]*)\]", repr(gc))
        ticks = (
            [int(x) for x in m.group(1).split(",")]
            if m and m.group(1).strip()
            else []
        )
        for p, t in [(i, t) for i, t in enumerate(ticks) if t > 0]:
            nop = self.nc.sync.nop(nofuse=True, hint="drain_split")
            sc = ScopedClock({})
            sc.require_at_least(None, p, t)
            wait_clock.add_sem_waits(nop.ins, sc)
        self.nc.sync.drain()
        self.nc.all_engine_barrier()
        assert self.sems is not None
        popped = self.nc._tile_sem_poison_stack.pop()
        assert popped is self._sem_poison
        self.nc.clear_and_free_semaphores(list(self.sems.allocated().values()))
        self.nc.all_engine_barrier()

    def __exit__(self, *args):
        r = super().__exit__(*args)
        _legalize_waits(self.nc)
        return r


# ---------------------------------------------------------------------------
# Device program (identical on all 8 cores).
# ---------------------------------------------------------------------------


def _chunk_counts(variant):
    return [1, 2, 3, 4, 5, 6, 7, 8] if variant == "causal" else [8] * 8


def _pair_schedule(variant, counts):
    """Per slot: list of chunk-pairs (a, b) (b may be None). The masked
    chunk (causal) is ordered first; slots run in ascending size so the
    tail after the last slot's PE work is only one final chain."""
    slots = []
    for slot in range(8):
        nch = counts[slot]
        if variant == "causal":
            order = [nch - 1] + list(range(nch - 1))
        else:
            order = list(range(nch))
        pairs = [
            (order[i], order[i + 1] if i + 1 < nch else None)
            for i in range(0, nch, 2)
        ]
        slots.append((slot, nch, pairs))
    return slots


def build_program(variant, repeat=1):
    """variant: 'causal' (slot i gets i+1 key-chunks of 256, one shared
    boundary mask), 'nomask' (8 chunks, no masks), 'generic' (8 chunks,
    per-chunk-pair masks streamed from DRAM). repeat: run the whole
    body N times (timing aid; output identical)."""
    counts = _chunk_counts(variant)
    npairs_tot = sum((c + 1) // 2 for c in counts)

    nc = bass.Bass("TRN2", target_bir_lowering=False, debug=False)
    wq8 = nc.declare_dram_parameter("wq8", [128, 8, E], FP8, isOutput=False)
    xq8 = nc.declare_dram_parameter("xq8", [128, 8, 1024], FP8, isOutput=False)
    xk8 = nc.declare_dram_parameter("xk8", [128, 8, S], FP8, isOutput=False)
    xv8 = nc.declare_dram_parameter("xv8", [128, 8, S], FP8, isOutput=False)
    bq8s = nc.declare_dram_parameter("bq8s", [128, 8], F32, isOutput=False)
    bqb1 = nc.declare_dram_parameter("bqb1", [128, E], F32, isOutput=False)
    ones8 = nc.declare_dram_parameter("ones8", [128, 2, 1], FP8, isOutput=False)
    id240 = nc.declare_dram_parameter("id240", [128, 128], FP8, isOutput=False)
    if variant == "causal":
        # fp8 residuals of xv-head / W for the high-precision V head
        # (output rows 0..255 see V almost unaveraged, so slot 0 uses a
        # residual-corrected bf16 V and bf16 P)
        wqlo8 = nc.declare_dram_parameter("wqlo8", [128, 8, E], FP8, isOutput=False)
        xvlo8 = nc.declare_dram_parameter("xvlo8", [128, 8, 256], FP8, isOutput=False)
        onesb = nc.declare_dram_parameter("onesb", [128, 1], BF16, isOutput=False)
        mask2 = nc.declare_dram_parameter("mask2", [128, 256], FP8, isOutput=False)
    elif variant == "generic":
        maskd = nc.declare_dram_parameter(
            "maskd", [npairs_tot, 128, 512], FP8, isOutput=False
        )
    out = nc.declare_dram_parameter("out", [1024, E], F32, isOutput=True)

    with _TileContext(nc) as tc:
        with (
            tc.tile_pool(name="const", bufs=1) as cpool,
            tc.tile_pool(name="big", bufs=1) as bpool,
        ):
            for _rep in range(repeat):
                wq_ctx = tc.tile_pool(name=f"wqpool{_rep}", bufs=1)
                wqpool = wq_ctx.__enter__()
                # Inputs land as chunk tiles in first-use order (the cost
                # model serializes transfers on one DMA-engine pool), so
                # each projection chunk's input precedes its matmuls.
                wq_p = [None] * 4
                xk_c = [None] * 4
                xq_c = [None] * 2
                xv_c = [None] * 4

                def _wq(dtp):
                    w = wqpool.tile([128, 2, E], FP8, tag=f"wq{dtp}", name="w")
                    nc.sync.dma_start(w[:], wq8[:, 2 * dtp : 2 * dtp + 2, :])
                    wq_p[dtp] = w

                def _xc(lst, src, t, ch):
                    x = bpool.tile([128, 8, 512], FP8, tag=f"{t}{ch}", name="x")
                    nc.sync.dma_start(x[:], src[:, :, ch * 512 : (ch + 1) * 512])
                    lst[ch] = x

                _wq(0)
                _xc(xk_c, xk8, "xk", 0)
                _wq(1)
                _wq(2)
                _wq(3)
                for ch in range(1, 4):
                    _xc(xk_c, xk8, "xk", ch)
                _xc(xq_c, xq8, "xq", 0)
                _xc(xq_c, xq8, "xq", 1)
                if _rep == 0:
                    bq8_sb = cpool.tile([128, 8], F32, tag="bq8")
                    nc.sync.dma_start(bq8_sb[:], bq8s[:])
                for ch in range(4):
                    _xc(xv_c, xv8, "xv", ch)
                if variant == "causal":
                    wqlo_sb = wqpool.tile([128, 8, E], FP8, tag="wqlo",
                                          name="wqlo_sb")
                    nc.sync.dma_start(wqlo_sb[:], wqlo8[:])
                    xvlo_sb = wqpool.tile([128, 8, 256], FP8, tag="xvlo",
                                          name="xvlo_sb")
                    nc.sync.dma_start(xvlo_sb[:], xvlo8[:])
                if _rep == 0:
                    bqb_sb = cpool.tile([128, E], F32, tag="bqb")
                    nc.sync.dma_start(bqb_sb[:], bqb1[:])
                    ones_sb = cpool.tile([128, 2, 1], FP8, tag="ones")
                    nc.sync.dma_start(ones_sb[:], ones8[:])
                    id_sb = cpool.tile([128, 128], FP8, tag="id240")
                    nc.sync.dma_start(id_sb[:], id240[:])
                    if variant == "causal":
                        onesb_sb = cpool.tile([128, 1], BF16, tag="onesb")
                        nc.sync.dma_start(onesb_sb[:], onesb[:])
                        mask_sb = cpool.tile([128, 256], FP8, tag="mask2")
                        nc.sync.dma_start(mask_sb[:], mask2[:])

                q_sb = bpool.tile([128, 8, 1024], FP8, tag="q")
                k_sb = bpool.tile([128, 8, S], FP8, tag="k")
                v_sb = bpool.tile([128, 16, E], FP8, tag="v")
                if variant == "causal":
                    v_bf = bpool.tile([128, 2, E], BF16, tag="vbf")

                # ---- projections ----
                # Each PSUM group is one [128,2048] tile (4 banks, two
                # groups in flight). Both PSUM-capable lanes (Act, DVE;
                # GPSIMD cannot read PSUM) evict each group in parallel
                # halves, so eviction wall time (~1.3us) stays under the
                # group's matmul time (~1.7us).
                with tc.tile_pool(name=f"projps{_rep}", bufs=2, space="PSUM") as ppsum:
                    # K^T then Q^T (e-major): out[e,s] += (wq8[d,e]).T@x[d,s]
                    for x_c, dst, nch, with_bias in (
                        (xk_c, k_sb, 4, False),
                        (xq_c, q_sb, 2, True),
                    ):
                        for ch in range(nch):
                            for g in range(2):
                                pss = ppsum.tile(
                                    [128, 2048], F32, tag="pp", name="pss"
                                )
                                for dtp in range(4):
                                    xslc = x_c[ch][:, 2 * dtp : 2 * dtp + 2, :]
                                    for ei in range(4):
                                        et = g * 4 + ei
                                        nc.tensor.matmul(
                                            pss[:, ei * 512 : (ei + 1) * 512],
                                            wq_p[dtp][
                                                :, :, et * 128 : (et + 1) * 128
                                            ],
                                            xslc,
                                            start=(dtp == 0),
                                            stop=(dtp == 3),
                                            perf_mode=DR,
                                        )
                                if with_bias:
                                    # Act: ei 0,1 (activation bias);
                                    # DVE: ei 2,3 (tensor_scalar w/ bias AP)
                                    for ei in range(4):
                                        et = g * 4 + ei
                                        dslc = dst[
                                            :, et, ch * 512 : (ch + 1) * 512
                                        ]
                                        pslc = pss[:, ei * 512 : (ei + 1) * 512]
                                        if ei < 2:
                                            nc.scalar.activation(
                                                dslc, pslc, AF.Identity,
                                                bias=bq8_sb[:, et : et + 1],
                                                scale=EVICT_SCALE,
                                            )
                                        else:
                                            nc.vector.tensor_scalar(
                                                dslc, pslc, EVICT_SCALE,
                                                bq8_sb[:, et : et + 1],
                                                ALU.mult, ALU.add,
                                            )
                                else:
                                    nc.scalar.activation(
                                        dst[
                                            :, g * 4 : g * 4 + 2,
                                            ch * 512 : (ch + 1) * 512,
                                        ],
                                        pss[:, 0:1024],
                                        AF.Identity, scale=EVICT_SCALE,
                                    )
                                    nc.vector.tensor_scalar_mul(
                                        dst[
                                            :, g * 4 + 2 : g * 4 + 4,
                                            ch * 512 : (ch + 1) * 512,
                                        ],
                                        pss[:, 1024:2048],
                                        EVICT_SCALE,
                                    )

                    # V (s-major): out[s, e] += x[d, s].T @ wq8[d, e]
                    for sb4 in range(4):
                        for g in range(2):
                            pss = ppsum.tile(
                                [128, 2048], F32, tag="pp", name="pss"
                            )
                            head = variant == "causal" and sb4 == 0 and g == 0
                            for dtp in range(4):
                                for s2 in range(2):
                                    si = g * 2 + s2
                                    for ec in range(2):
                                        nc.tensor.matmul(
                                            pss[
                                                :,
                                                (s2 * 2 + ec) * 512
                                                : (s2 * 2 + ec + 1) * 512,
                                            ],
                                            xv_c[sb4][
                                                :, 2 * dtp : 2 * dtp + 2,
                                                si * 128 : (si + 1) * 128,
                                            ],
                                            wq_p[dtp][
                                                :, :, ec * 512 : (ec + 1) * 512
                                            ],
                                            start=(dtp == 0),
                                            stop=(dtp == 3) and not head,
                                            perf_mode=DR,
                                        )
                            if head:
                                # residual passes accumulate rx@W8 + x8@rW
                                # into the same PSUM: V head reaches ~bf16
                                # accuracy (residuals are small, their fp8
                                # quantization error is ~2^-10 absolute)
                                for dtp in range(4):
                                    for si in range(2):
                                        for ec in range(2):
                                            cslc = pss[
                                                :,
                                                (si * 2 + ec) * 512
                                                : (si * 2 + ec + 1) * 512,
                                            ]
                                            nc.tensor.matmul(
                                                cslc,
                                                xvlo_sb[
                                                    :, 2 * dtp : 2 * dtp + 2,
                                                    si * 128 : (si + 1) * 128,
                                                ],
                                                wq_p[dtp][
                                                    :, :,
                                                    ec * 512 : (ec + 1) * 512,
                                                ],
                                                start=False, stop=False,
                                                perf_mode=DR,
                                            )
                                            nc.tensor.matmul(
                                                cslc,
                                                xv_c[0][
                                                    :, 2 * dtp : 2 * dtp + 2,
                                                    si * 128 : (si + 1) * 128,
                                                ],
                                                wqlo_sb[
                                                    :, 2 * dtp : 2 * dtp + 2,
                                                    ec * 512 : (ec + 1) * 512,
                                                ],
                                                start=False,
                                                stop=(dtp == 3),
                                                perf_mode=DR,
                                            )
                                nc.scalar.activation(
                                    v_bf[:, 0, :], pss[:, 0:1024],
                                    AF.Copy,
                                )
                                nc.vector.tensor_copy(
                                    v_bf[:, 1, :], pss[:, 1024:2048],
                                )
                            nc.scalar.activation(
                                v_sb[:, sb4 * 4 + g * 2, :],
                                pss[:, 0:1024],
                                AF.Identity, scale=EVICT_SCALE,
                            )
                            nc.vector.tensor_scalar_mul(
                                v_sb[:, sb4 * 4 + g * 2 + 1, :],
                                pss[:, 1024:2048],
                                EVICT_SCALE,
                            )

                # ---- attention (k-major, chunk pairs) ----
                wq_ctx.__exit__(None, None, None)
                with (
                    tc.tile_pool(name=f"pt{_rep}", bufs=3) as ptpool,
                    tc.tile_pool(name=f"small{_rep}", bufs=4) as spool,
                    tc.tile_pool(name=f"mstage{_rep}", bufs=2) as mpool,
                    tc.tile_pool(name=f"opool{_rep}", bufs=2) as opool,
                    tc.tile_pool(name=f"sps{_rep}", bufs=2, space="PSUM") as spsum,
                    tc.tile_pool(name=f"rps{_rep}", bufs=2, space="PSUM") as rpsum,
                    tc.tile_pool(name=f"ops{_rep}", bufs=2, space="PSUM") as opsum,
                ):
                    slot_ps = {}
                    pair_i = [0]

                    def emit_scores(slot, pairs, pi):
                        a, b = pairs[pi]
                        if variant == "causal":
                            masked_cols = [0, 128] if pi == 0 else []
                            msk = mask_sb
                        elif variant == "generic":
                            masked_cols = [0, 128] + ([256, 384] if b is not None else [])
                            msk = mpool.tile([128, 512], FP8, tag="msk", name="m")
                            nc.sync.dma_start(msk[:], maskd[pair_i[0]])
                        else:
                            masked_cols = []
                        pair_i[0] += 1
                        s_ps = spsum.tile([128, 512], F32, tag="s", name="s_ps")
                        for half, c in enumerate((a, b)):
                            if c is None:
                                continue
                            for kt in range(2):
                                kk = 2 * c + kt
                                col = half * 256 + kt * 128
                                has_mask = col in masked_cols
                                for etp in range(4):
                                    nc.tensor.matmul(
                                        s_ps[:, col : col + 128],
                                        k_sb[:, 2 * etp : 2 * etp + 2,
                                             kk * 128 : (kk + 1) * 128],
                                        q_sb[:, 2 * etp : 2 * etp + 2,
                                             slot * 128 : (slot + 1) * 128],
                                        start=(etp == 0),
                                        stop=(etp == 3) and not has_mask,
                                        perf_mode=DR,
                                    )
                                if has_mask:
                                    # accumulate 240*mask8 (= -57600 on
                                    # masked positions) on the PE itself
                                    nc.tensor.matmul(
                                        s_ps[:, col : col + 128],
                                        id_sb[:],
                                        msk[:, col : col + 128],
                                        start=False, stop=True,
                                    )
                        wid = 512 if b is not None else 256
                        if variant == "causal" and slot == 0:
                            # short output rows: bf16 P (and bf16 V) for
                            # the only chunk slot 0 attends
                            psT = ptpool.tile([128, 2, 128], BF16,
                                              tag="ptbf", name="psTb")
                            nc.scalar.activation(
                                psT[:], s_ps[:, 0:256], AF.Exp,
                                scale=EXP_SCALE,
                            )
                        else:
                            psT = ptpool.tile([128, 4, 128], FP8, tag="pt",
                                              name="psT")
                            nc.scalar.activation(
                                psT[:, 0 : wid // 128, :], s_ps[:, 0:wid],
                                AF.Exp, scale=EXP_SCALE,
                            )
                        return psT

                    def emit_pv(slot, nch, pairs, pi, psT):
                        for half, c in enumerate(pairs[pi]):
                            if c is None:
                                continue
                            pos = 2 * pi + half
                            first = pos == 0
                            last = pos == nch - 1
                            if first:
                                o_t = opsum.tile(
                                    [128, 1024], F32, tag="o", name="o_t"
                                )
                                rs_t = rpsum.tile(
                                    [128, 1], F32, tag="rs", name="rs_t"
                                )
                                slot_ps[slot] = (o_t, rs_t)
                            o_ps, rs_ps = slot_ps[slot]
                            if variant == "causal" and slot == 0:
                                # bf16 PV over the residual-corrected V
                                # head (no DoubleRow for bf16)
                                for kt in range(2):
                                    s_, e_ = kt == 0, kt == 1
                                    nc.tensor.matmul(
                                        rs_ps[:], psT[:, kt, :], onesb_sb[:],
                                        start=s_, stop=e_,
                                    )
                                    for ec in range(2):
                                        nc.tensor.matmul(
                                            o_ps[:, ec * 512 : (ec + 1) * 512],
                                            psT[:, kt, :],
                                            v_bf[:, kt,
                                                 ec * 512 : (ec + 1) * 512],
                                            start=s_, stop=e_,
                                        )
                            else:
                                pslc = psT[:, 2 * half : 2 * half + 2, :]
                                nc.tensor.matmul(
                                    rs_ps[:], pslc, ones_sb[:],
                                    start=first, stop=last, perf_mode=DR,
                                )
                                for ec in range(2):
                                    nc.tensor.matmul(
                                        o_ps[:, ec * 512 : (ec + 1) * 512],
                                        pslc,
                                        v_sb[:, 2 * c : 2 * c + 2,
                                             ec * 512 : (ec + 1) * 512],
                                        start=first, stop=last, perf_mode=DR,
                                    )
                            if last:
                                rcp = spool.tile([128, 1], F32, tag="rcp",
                                                 name="rcp")
                                nc.vector.reciprocal(rcp[:], rs_ps[:])
                                o_sb = opool.tile([128, E], F32, tag="osb",
                                                  name="o_sb")
                                # out = o_ps/(8*rowsum) + bq in one pass
                                nc.vector.scalar_tensor_tensor(
                                    o_sb[:], o_ps[:], rcp[:],
                                    bqb_sb[:], ALU.mult, ALU.add,
                                )
                                nc.sync.dma_start(
                                    out[slot * 128 : (slot + 1) * 128, :],
                                    o_sb[:],
                                )

                    # PV lags scores by two pairs so the exp latency
                    # never stalls the PE
                    pending = deque()
                    for slot, nch, pairs in _pair_schedule(variant, counts):
                        for pi in range(len(pairs)):
                            psT = emit_scores(slot, pairs, pi)
                            pending.append((slot, nch, pairs, pi, psT))
                            if len(pending) > 2:
                                emit_pv(*pending.popleft())
                    while pending:
                        emit_pv(*pending.popleft())

    return nc


# ---------------------------------------------------------------------------
# Host wrapper.
# ---------------------------------------------------------------------------

_prog_cache = {}


def _get_program(variant):
    if variant not in _prog_cache:
        _prog_cache[variant] = build_program(variant)
    return _prog_cache[variant]


def _analyze_mask(att_mask):
    causal = np.array_equal(
        att_mask, np.triu(np.ones((S, S), dtype=att_mask.dtype), 1)
    )
    if causal:
        return "causal"
    if not att_mask.any():
        return "nomask"
    return "generic"


def _dmajor(x):
    """[s, d] f32 -> contiguous [128, 8, s] fp8 (d split as dt*128+dp)."""
    return _dmajor_raw(x.astype(NP_FP8))


def _dmajor_raw(x8):
    """[s, d] fp8 -> contiguous [128, 8, s]."""
    xT = np.ascontiguousarray(x8.T)  # [d, s]
    return np.ascontiguousarray(xT.reshape(8, 128, -1).transpose(1, 0, 2))


def _causal_mask2(h):
    """[128, 256] fp8 k-major boundary mask (0 keep / -240 drop) for
    half h: [diag|full] for even tiles, [zero|diag] for odd."""
    kk = np.arange(128)[:, None]
    qq = np.arange(128)[None, :]
    diag = np.where(kk > qq, -MASK_VAL, 0.0)
    full = np.full((128, 128), -MASK_VAL)
    zero = np.zeros((128, 128))
    pair = (diag, full) if h == 0 else (zero, diag)
    return np.ascontiguousarray(np.concatenate(pair, axis=1)).astype(NP_FP8)


def _build_in_maps(xq, xk, xv, Wq, bq, att_mask, variant):
    xq = np.asarray(xq, dtype=np.float32)
    xk = np.asarray(xk, dtype=np.float32)
    xv = np.asarray(xv, dtype=np.float32)
    Wq = np.asarray(Wq, dtype=np.float32)
    bq = np.asarray(bq, dtype=np.float32)
    att_mask = np.asarray(att_mask)

    wqT32 = Wq.T * W_SCALE
    wq8_flat = wqT32.astype(NP_FP8)
    wq8 = np.ascontiguousarray(
        wq8_flat.reshape(8, 128, E).transpose(1, 0, 2)
    )
    wqlo8 = np.ascontiguousarray(
        (wqT32 - wq8_flat.astype(np.float32))
        .astype(NP_FP8).reshape(8, 128, E).transpose(1, 0, 2)
    )
    onesb = np.full((128, 1), 32.0, dtype=NP_BF16)
    bq8s = np.ascontiguousarray(bq.reshape(8, 128).T * QKV_SCALE)
    bqb1 = np.ascontiguousarray(np.broadcast_to(bq, (128, E))).astype(
        np.float32
    )
    ones8 = np.full((128, 2, 1), QKV_SCALE, dtype=NP_FP8)
    id240 = (np.eye(128) * MASK_VAL).astype(NP_FP8)
    counts = _chunk_counts(variant)
    tiles_by_half = (TILES_H0, TILES_H1)

    in_maps = []
    for c in range(NCORES):
        b, h = divmod(c, 2)
        tiles = tiles_by_half[h]
        rows = np.concatenate(
            [np.arange(t * 128, (t + 1) * 128) for t in tiles]
        )
        m = {
            "wq8": wq8,
            "xq8": _dmajor(xq[b][rows]),
            "xk8": _dmajor(xk[b]),
            "xv8": _dmajor(xv[b]),
            "bq8s": bq8s,
            "bqb1": bqb1,
            "ones8": ones8,
            "id240": id240,
        }
        if variant == "causal":
            m["mask2"] = _causal_mask2(h)
            m["wqlo8"] = wqlo8
            m["onesb"] = onesb
            xvh = xv[b][:256]  # head keys, same fp8 base as xv8
            rxv = xvh - xvh.astype(NP_FP8).astype(np.float32)
            m["xvlo8"] = _dmajor_raw(rxv.astype(NP_FP8))
        elif variant == "generic":
            mds = []
            for slot, nch, pairs in _pair_schedule(variant, counts):
                t = tiles[slot]
                for a, bch in pairs:
                    md = np.zeros((128, 512))
                    for half, ch in enumerate((a, bch)):
                        if ch is None:
                            continue
                        blk = att_mask[
                            t * 128 : (t + 1) * 128,
                            ch * 256 : (ch + 1) * 256,
                        ]  # [q, k]
                        kmaj = blk.T.astype(np.float64) * -MASK_VAL
                        md[:, half * 256 : half * 256 + 128] = kmaj[:128]
                        md[:, half * 256 + 128 : (half + 1) * 256] = kmaj[128:]
                    mds.append(md.astype(NP_FP8))
            m["maskd"] = np.stack(mds)
        in_maps.append(m)
    return in_maps


def kernel(xq, xk, xv, Wq, bq, att_mask):
    from concourse.bass_utils import run_bass_kernel_spmd

    variant = _analyze_mask(np.asarray(att_mask))
    nc = _get_program(variant)
    in_maps = _build_in_maps(xq, xk, xv, Wq, bq, att_mask, variant)

    res = run_bass_kernel_spmd(nc, in_maps, list(range(NCORES)))

    tiles_by_half = (TILES_H0, TILES_H1)
    out = np.empty((B, S, E), dtype=np.float32)
    for c in range(NCORES):
        b, h = divmod(c, 2)
        tiles = tiles_by_half[h]
        oc = res.results[c]["out"]
        for i, t in enumerate(tiles):
            out[b, t * 128 : (t + 1) * 128, :] = oc[i * 128 : (i + 1) * 128, :]
    return out


# revision 65
# speedup vs baseline: 3.0650x; 1.0193x over previous
"""Single-head attention (shared QKV weight) on 8 Trainium2 NeuronCores.

Problem: B=4, S=2048, D=E=1024
  Q = xq@Wq.T + bq ; K = xk@Wq.T + bq ; V = xv@Wq.T + bq
  out = softmax(mask(Q@K.T/sqrt(E))) @ V

Sharding: data-parallel over batch x query-halves -> 8 cores. Core c
handles batch b=c//2 and half h=c%2 of the query tiles: even global
128-row tiles for h=0, odd for h=1, so slot i on every core processes
exactly i+1 key-chunks of 256 under the causal mask (identical SPMD
instruction stream, balanced FLOPs). Each core computes the full K/V
projection of its batch and its own Q projection.

All matmuls run in fp8e4 (e4m3) with DoubleRow perf mode: two 128-deep
contraction tiles per instruction at 0.5 PE cycles/row -> 4x fp32r
throughput. Scaling keeps everything in fp8 range:
- host supplies x (unit-normal, as-is) and 32*Wq in fp8
- projections evict PSUM/4 -> fp8 (values = 8x true Q/K/V)
- scores PSUM = 64 * QK^T_true; exp applies 1/(64*32) = 1/2048
- attention is k-major: scores land as S^T[k,q], exp writes fp8
  P^T[k,2,q] tiles directly usable as the PV stationary operand (no
  transposes); row-sums come from an N=1 matmul with an 8.0-vector
  (so rs = 8*rowsum matches o_ps = 8*P@V); the final
  out = o_ps*(1/rs) + bq is one scalar_tensor_tensor op.
- the causal boundary mask is applied ON the PE: an extra matmul per
  masked k-tile accumulates 240 * mask8[k,q] (mask8 in {0,-240}) =
  -57600 into the raw-score PSUM (|s| < 5e3), so exp -> 0 with no
  cross-engine hop before the exp.

Only Act and DVE can read PSUM (GPSIMD cannot), so each projection
PSUM group (one [128,2048] tile, 4 banks) is evicted by both lanes in
parallel (two [128,1024] halves), keeping per-group eviction wall time
under the group's matmul time.

Math shortcuts (exact): K-bias cancels in softmax; Q-bias fused into
the Q eviction; V-bias added via the fused final op (softmax rows sum
to 1); scores bounded (|s|/32 <~ 2) so softmax skips max-subtraction.
"""

import re
from collections import deque

import numpy as np
import ml_dtypes

import concourse.bass as bass
import concourse.mybir as mybir
import concourse.tile as tile
from concourse.vector_clock import ScopedClock

F32 = mybir.dt.float32
FP8 = mybir.dt.float8e4
BF16 = mybir.dt.bfloat16
NP_FP8 = ml_dtypes.float8_e4m3
NP_BF16 = ml_dtypes.bfloat16
AF = mybir.ActivationFunctionType
ALU = mybir.AluOpType
DR = mybir.MatmulPerfMode.DoubleRow

B, S, D, E = 4, 2048, 1024, 1024
NCORES = 8
W_SCALE = 32.0  # host multiplies Wq by this before fp8 conversion
QKV_SCALE = 8.0  # fp8 Q/K/V values are 8x the true values
EVICT_SCALE = QKV_SCALE / W_SCALE  # PSUM -> fp8 eviction multiplier
EXP_SCALE = (1.0 / 32.0) / (QKV_SCALE * QKV_SCALE)  # softmax scale / 64
MASK_VAL = 240.0  # ident240 @ mask8(-240) adds -57600 to raw scores

# Half h owns global q-tiles h, h+2, ..., h+14. Slot i (its i-th local
# 128-row tile) is global tile 2i+h and needs ceil((2i+h+1)/2) = i+1
# chunks of 256 keys -> identical instruction stream on all cores.
TILES_H0 = [0, 2, 4, 6, 8, 10, 12, 14]
TILES_H1 = [1, 3, 5, 7, 9, 11, 13, 15]

# ---------------------------------------------------------------------------
# Workarounds for this container's walrus build, which rejects any
# instruction carrying more than one semaphore wait.
# ---------------------------------------------------------------------------

_split_counter = [0]


def _legalize_waits(nc):
    """Move all-but-one sem wait from each instruction onto single-wait
    NoOps inserted immediately before it on the same engine. Engines
    dispatch in order, so the nops' waits are satisfied before the
    instruction issues."""
    for f in nc.m.functions:
        for bb in f.blocks:
            insts = list(bb.instructions)
            out = []
            changed = False
            for inst in insts:
                si = inst.sync_info
                if si is not None and si.on_wait is not None and len(si.on_wait) > 1:
                    waits = list(si.on_wait)
                    for w in waits[:-1]:
                        _split_counter[0] += 1
                        nop = mybir.InstNoOp(
                            name=f"I-waitsplit-{_split_counter[0]}",
                            opcode="NoOp",
                            engine=inst.engine,
                            sync_info=mybir.SyncInfo(on_wait=[w], on_update=[]),
                        )
                        nc.register_instruction(nop)
                        out.append(nop)
                    si.on_wait = [waits[-1]]
                    changed = True
                out.append(inst)
            if changed:
                bb.instructions = out


class _TileContext(tile.TileContext):
    def __init__(self, nc, **kw):
        kw.setdefault("pool_alloc_mode", "queue")
        super().__init__(nc, **kw)

    def _drain_and_barrier(self, tick_clock, wait_clock):
        gc = tick_clock.global_clock
        m = re.search(r"\[([0-9, ]*)\]", repr(gc))
        ticks = (
            [int(x) for x in m.group(1).split(",")]
            if m and m.group(1).strip()
            else []
        )
        for p, t in [(i, t) for i, t in enumerate(ticks) if t > 0]:
            nop = self.nc.sync.nop(nofuse=True, hint="drain_split")
            sc = ScopedClock({})
            sc.require_at_least(None, p, t)
            wait_clock.add_sem_waits(nop.ins, sc)
        self.nc.sync.drain()
        self.nc.all_engine_barrier()
        assert self.sems is not None
        popped = self.nc._tile_sem_poison_stack.pop()
        assert popped is self._sem_poison
        self.nc.clear_and_free_semaphores(list(self.sems.allocated().values()))
        self.nc.all_engine_barrier()

    def __exit__(self, *args):
        r = super().__exit__(*args)
        _legalize_waits(self.nc)
        return r


# ---------------------------------------------------------------------------
# Device program (identical on all 8 cores).
# ---------------------------------------------------------------------------


def _chunk_counts(variant):
    return [1, 2, 3, 4, 5, 6, 7, 8] if variant == "causal" else [8] * 8


def _pair_schedule(variant, counts):
    """Per slot: list of chunk-pairs (a, b) (b may be None). The masked
    chunk (causal) is ordered first; slots run in ascending size so the
    tail after the last slot's PE work is only one final chain."""
    slots = []
    for slot in range(8):
        nch = counts[slot]
        if variant == "causal":
            order = [nch - 1] + list(range(nch - 1))
        else:
            order = list(range(nch))
        pairs = [
            (order[i], order[i + 1] if i + 1 < nch else None)
            for i in range(0, nch, 2)
        ]
        slots.append((slot, nch, pairs))
    return slots


def build_program(variant, repeat=1):
    """variant: 'causal' (slot i gets i+1 key-chunks of 256, one shared
    boundary mask), 'nomask' (8 chunks, no masks), 'generic' (8 chunks,
    per-chunk-pair masks streamed from DRAM). repeat: run the whole
    body N times (timing aid; output identical)."""
    counts = _chunk_counts(variant)
    npairs_tot = sum((c + 1) // 2 for c in counts)

    nc = bass.Bass("TRN2", target_bir_lowering=False, debug=False)
    wq8 = nc.declare_dram_parameter("wq8", [128, 8, E], FP8, isOutput=False)
    xq8 = nc.declare_dram_parameter("xq8", [128, 8, 1024], FP8, isOutput=False)
    xk8 = nc.declare_dram_parameter("xk8", [128, 8, S], FP8, isOutput=False)
    xv8 = nc.declare_dram_parameter("xv8", [128, 8, S], FP8, isOutput=False)
    bq8s = nc.declare_dram_parameter("bq8s", [128, 8], F32, isOutput=False)
    bqb1 = nc.declare_dram_parameter("bqb1", [128, E], F32, isOutput=False)
    ones8 = nc.declare_dram_parameter("ones8", [128, 2, 1], FP8, isOutput=False)
    id240 = nc.declare_dram_parameter("id240", [128, 128], FP8, isOutput=False)
    if variant == "causal":
        # fp8 residuals of xv-head / W for the high-precision V head
        # (output rows 0..255 see V almost unaveraged, so slot 0 uses a
        # residual-corrected bf16 V and bf16 P)
        wqlo8 = nc.declare_dram_parameter("wqlo8", [128, 8, E], FP8, isOutput=False)
        xvlo8 = nc.declare_dram_parameter("xvlo8", [128, 8, 256], FP8, isOutput=False)
        onesb = nc.declare_dram_parameter("onesb", [128, 1], BF16, isOutput=False)
        mask2 = nc.declare_dram_parameter("mask2", [128, 256], FP8, isOutput=False)
    elif variant == "generic":
        maskd = nc.declare_dram_parameter(
            "maskd", [npairs_tot, 128, 512], FP8, isOutput=False
        )
    out = nc.declare_dram_parameter("out", [1024, E], F32, isOutput=True)

    with _TileContext(nc) as tc:
        with (
            tc.tile_pool(name="const", bufs=1) as cpool,
            tc.tile_pool(name="big", bufs=1) as bpool,
        ):
            for _rep in range(repeat):
                wq_ctx = tc.tile_pool(name=f"wqpool{_rep}", bufs=1)
                wqpool = wq_ctx.__enter__()
                # Inputs land as chunk tiles in first-use order (the cost
                # model serializes transfers on one DMA-engine pool), so
                # each projection chunk's input precedes its matmuls.
                def _xt(src, t, c0, nch):
                    x = bpool.tile([128, 8, 512 * nch], FP8, tag=t, name="x")
                    nc.sync.dma_start(
                        x[:], src[:, :, c0 * 512 : (c0 + nch) * 512]
                    )
                    return x

                # DMA sizes graded so each projection chunk's input lands
                # just before the PE reaches it on the serialized DMA pool
                wq_t = [None] * 4

                def _wq(dtp):
                    w = wqpool.tile([128, 2, E], FP8, tag=f"wq{dtp}", name="w")
                    nc.sync.dma_start(w[:], wq8[:, 2 * dtp : 2 * dtp + 2, :])
                    wq_t[dtp] = w

                def wq_slc(dtp, lo, hi):
                    return wq_t[dtp][:, :, lo:hi]

                _wq(0)
                xk_t0 = _xt(xk8, "xk0", 0, 1)
                _wq(1)
                _wq(2)
                _wq(3)
                xk_t1 = _xt(xk8, "xk1", 1, 1)
                xk_t2 = _xt(xk8, "xk2", 2, 2)
                xq_sb = _xt(xq8, "xq", 0, 2)
                if _rep == 0:
                    bq8_sb = cpool.tile([128, 8], F32, tag="bq8")
                    nc.sync.dma_start(bq8_sb[:], bq8s[:])
                xv_t0 = _xt(xv8, "xv0", 0, 2)
                xv_t1 = _xt(xv8, "xv1", 2, 2)

                def xk_slc(ch, dtp):
                    if ch < 2:
                        t, off = (xk_t0, xk_t1)[ch], 0
                    else:
                        t, off = xk_t2, (ch - 2) * 512
                    return t[:, 2 * dtp : 2 * dtp + 2, off : off + 512]

                def xq_slc(ch, dtp):
                    return xq_sb[
                        :, 2 * dtp : 2 * dtp + 2, ch * 512 : (ch + 1) * 512
                    ]

                def xv_slc(sb4, dtp, lo, hi):
                    t = (xv_t0, xv_t1)[sb4 // 2]
                    off = (sb4 % 2) * 512
                    return t[:, 2 * dtp : 2 * dtp + 2, off + lo : off + hi]
                if variant == "causal":
                    wqlo_sb = wqpool.tile([128, 8, E], FP8, tag="wqlo",
                                          name="wqlo_sb")
                    nc.sync.dma_start(wqlo_sb[:], wqlo8[:])
                    xvlo_sb = wqpool.tile([128, 8, 256], FP8, tag="xvlo",
                                          name="xvlo_sb")
                    nc.sync.dma_start(xvlo_sb[:], xvlo8[:])
                if _rep == 0:
                    bqb_sb = cpool.tile([128, E], F32, tag="bqb")
                    nc.sync.dma_start(bqb_sb[:], bqb1[:])
                    ones_sb = cpool.tile([128, 2, 1], FP8, tag="ones")
                    nc.sync.dma_start(ones_sb[:], ones8[:])
                    id_sb = cpool.tile([128, 128], FP8, tag="id240")
                    nc.sync.dma_start(id_sb[:], id240[:])
                    if variant == "causal":
                        onesb_sb = cpool.tile([128, 1], BF16, tag="onesb")
                        nc.sync.dma_start(onesb_sb[:], onesb[:])
                        mask_sb = cpool.tile([128, 256], FP8, tag="mask2")
                        nc.sync.dma_start(mask_sb[:], mask2[:])

                q_sb = bpool.tile([128, 8, 1024], FP8, tag="q")
                k_sb = bpool.tile([128, 8, S], FP8, tag="k")
                v_t = [
                    bpool.tile([128, 4, E], FP8, tag=f"v{i}", name="v")
                    for i in range(4)
                ]
                if variant == "causal":
                    v_bf = bpool.tile([128, 2, E], BF16, tag="vbf")

                # ---- projections ----
                # Each PSUM group is one [128,2048] tile (4 banks, two
                # groups in flight). Both PSUM-capable lanes (Act, DVE;
                # GPSIMD cannot read PSUM) evict each group in parallel
                # halves, so eviction wall time (~1.3us) stays under the
                # group's matmul time (~1.7us).
                with tc.tile_pool(name=f"projps{_rep}", bufs=2, space="PSUM") as ppsum:
                    # K^T then Q^T (e-major): out[e,s] += (wq8[d,e]).T@x[d,s]
                    for x_slc, dst, nch, with_bias in (
                        (xk_slc, k_sb, 4, False),
                        (xq_slc, q_sb, 2, True),
                    ):
                        for ch in range(nch):
                            for g in range(2):
                                pss = ppsum.tile(
                                    [128, 2048], F32, tag="pp", name="pss"
                                )
                                for dtp in range(4):
                                    xslc = x_slc(ch, dtp)
                                    for ei in range(4):
                                        et = g * 4 + ei
                                        nc.tensor.matmul(
                                            pss[:, ei * 512 : (ei + 1) * 512],
                                            wq_p[dtp][
                                                :, :, et * 128 : (et + 1) * 128
                                            ],
                                            xslc,
                                            start=(dtp == 0),
                                            stop=(dtp == 3),
                                            perf_mode=DR,
                                        )
                                if with_bias:
                                    # Act: ei 0,1 (activation bias);
                                    # DVE: ei 2,3 (tensor_scalar w/ bias AP)
                                    for ei in range(4):
                                        et = g * 4 + ei
                                        dslc = dst[
                                            :, et, ch * 512 : (ch + 1) * 512
                                        ]
                                        pslc = pss[:, ei * 512 : (ei + 1) * 512]
                                        if ei < 2:
                                            nc.scalar.activation(
                                                dslc, pslc, AF.Identity,
                                                bias=bq8_sb[:, et : et + 1],
                                                scale=EVICT_SCALE,
                                            )
                                        else:
                                            nc.vector.tensor_scalar(
                                                dslc, pslc, EVICT_SCALE,
                                                bq8_sb[:, et : et + 1],
                                                ALU.mult, ALU.add,
                                            )
                                else:
                                    nc.scalar.activation(
                                        dst[
                                            :, g * 4 : g * 4 + 3,
                                            ch * 512 : (ch + 1) * 512,
                                        ],
                                        pss[:, 0:1536],
                                        AF.Identity, scale=EVICT_SCALE,
                                    )
                                    nc.vector.tensor_scalar_mul(
                                        dst[
                                            :, g * 4 + 3 : g * 4 + 4,
                                            ch * 512 : (ch + 1) * 512,
                                        ],
                                        pss[:, 1536:2048],
                                        EVICT_SCALE,
                                    )

                    # V (s-major): out[s, e] += x[d, s].T @ wq8[d, e]
                    for sb4 in range(4):
                        for g in range(2):
                            pss = ppsum.tile(
                                [128, 2048], F32, tag="pp", name="pss"
                            )
                            head = variant == "causal" and sb4 == 0 and g == 0
                            for dtp in range(4):
                                for s2 in range(2):
                                    si = g * 2 + s2
                                    for ec in range(2):
                                        nc.tensor.matmul(
                                            pss[
                                                :,
                                                (s2 * 2 + ec) * 512
                                                : (s2 * 2 + ec + 1) * 512,
                                            ],
                                            xv_slc(
                                                sb4, dtp,
                                                si * 128, (si + 1) * 128,
                                            ),
                                            wq_p[dtp][
                                                :, :, ec * 512 : (ec + 1) * 512
                                            ],
                                            start=(dtp == 0),
                                            stop=(dtp == 3) and not head,
                                            perf_mode=DR,
                                        )
                            if head:
                                # residual passes accumulate rx@W8 + x8@rW
                                # into the same PSUM: V head reaches ~bf16
                                # accuracy (residuals are small, their fp8
                                # quantization error is ~2^-10 absolute)
                                for dtp in range(4):
                                    for si in range(2):
                                        for ec in range(2):
                                            cslc = pss[
                                                :,
                                                (si * 2 + ec) * 512
                                                : (si * 2 + ec + 1) * 512,
                                            ]
                                            nc.tensor.matmul(
                                                cslc,
                                                xvlo_sb[
                                                    :, 2 * dtp : 2 * dtp + 2,
                                                    si * 128 : (si + 1) * 128,
                                                ],
                                                wq_p[dtp][
                                                    :, :,
                                                    ec * 512 : (ec + 1) * 512,
                                                ],
                                                start=False, stop=False,
                                                perf_mode=DR,
                                            )
                                            nc.tensor.matmul(
                                                cslc,
                                                xv_slc(
                                                    0, dtp,
                                                    si * 128, (si + 1) * 128,
                                                ),
                                                wqlo_sb[
                                                    :, 2 * dtp : 2 * dtp + 2,
                                                    ec * 512 : (ec + 1) * 512,
                                                ],
                                                start=False,
                                                stop=(dtp == 3),
                                                perf_mode=DR,
                                            )
                                nc.scalar.activation(
                                    v_bf[:, 0, :], pss[:, 0:1024],
                                    AF.Copy,
                                )
                                nc.vector.tensor_copy(
                                    v_bf[:, 1, :], pss[:, 1024:2048],
                                )
                            nc.scalar.activation(
                                v_t[sb4][:, g * 2, :],
                                pss[:, 0:1024],
                                AF.Identity, scale=EVICT_SCALE,
                            )
                            nc.vector.tensor_scalar_mul(
                                v_t[sb4][:, g * 2 + 1, :],
                                pss[:, 1024:2048],
                                EVICT_SCALE,
                            )

                # ---- attention (k-major, chunk pairs) ----
                wq_ctx.__exit__(None, None, None)
                with (
                    tc.tile_pool(name=f"pt{_rep}", bufs=4) as ptpool,
                    tc.tile_pool(name=f"small{_rep}", bufs=4) as spool,
                    tc.tile_pool(name=f"mstage{_rep}", bufs=2) as mpool,
                    tc.tile_pool(name=f"opool{_rep}", bufs=2) as opool,
                    tc.tile_pool(name=f"sps{_rep}", bufs=3, space="PSUM") as spsum,
                    tc.tile_pool(name=f"rps{_rep}", bufs=1, space="PSUM") as rpsum,
                    tc.tile_pool(name=f"ops{_rep}", bufs=2, space="PSUM") as opsum,
                ):
                    slot_ps = {}
                    pair_i = [0]

                    def emit_scores(slot, pairs, pi):
                        a, b = pairs[pi]
                        if variant == "causal":
                            masked_cols = [0, 128] if pi == 0 else []
                            msk = mask_sb
                        elif variant == "generic":
                            masked_cols = [0, 128] + ([256, 384] if b is not None else [])
                            msk = mpool.tile([128, 512], FP8, tag="msk", name="m")
                            nc.sync.dma_start(msk[:], maskd[pair_i[0]])
                        else:
                            masked_cols = []
                        pair_i[0] += 1
                        s_ps = spsum.tile([128, 512], F32, tag="s", name="s_ps")
                        for half, c in enumerate((a, b)):
                            if c is None:
                                continue
                            for kt in range(2):
                                kk = 2 * c + kt
                                col = half * 256 + kt * 128
                                has_mask = col in masked_cols
                                for etp in range(4):
                                    nc.tensor.matmul(
                                        s_ps[:, col : col + 128],
                                        k_sb[:, 2 * etp : 2 * etp + 2,
                                             kk * 128 : (kk + 1) * 128],
                                        q_sb[:, 2 * etp : 2 * etp + 2,
                                             slot * 128 : (slot + 1) * 128],
                                        start=(etp == 0),
                                        stop=(etp == 3) and not has_mask,
                                        perf_mode=DR,
                                    )
                                if has_mask:
                                    # accumulate 240*mask8 (= -57600 on
                                    # masked positions) on the PE itself
                                    nc.tensor.matmul(
                                        s_ps[:, col : col + 128],
                                        id_sb[:],
                                        msk[:, col : col + 128],
                                        start=False, stop=True,
                                    )
                        wid = 512 if b is not None else 256
                        if variant == "causal" and slot == 0:
                            # short output rows: bf16 P (and bf16 V) for
                            # the only chunk slot 0 attends
                            psT = ptpool.tile([128, 2, 128], BF16,
                                              tag="ptbf", name="psTb")
                            nc.scalar.activation(
                                psT[:], s_ps[:, 0:256], AF.Exp,
                                scale=EXP_SCALE,
                            )
                        else:
                            psT = ptpool.tile([128, 4, 128], FP8, tag="pt",
                                              name="psT")
                            nc.scalar.activation(
                                psT[:, 0 : wid // 128, :], s_ps[:, 0:wid],
                                AF.Exp, scale=EXP_SCALE,
                            )
                        return psT

                    def emit_pv(slot, nch, pairs, pi, psT):
                        for half, c in enumerate(pairs[pi]):
                            if c is None:
                                continue
                            pos = 2 * pi + half
                            first = pos == 0
                            last = pos == nch - 1
                            if first:
                                o_t = opsum.tile(
                                    [128, 1024], F32, tag="o", name="o_t"
                                )
                                rs_t = rpsum.tile(
                                    [128, 1], F32, tag="rs", name="rs_t"
                                )
                                slot_ps[slot] = (o_t, rs_t)
                            o_ps, rs_ps = slot_ps[slot]
                            if variant == "causal" and slot == 0:
                                # bf16 PV over the residual-corrected V
                                # head (no DoubleRow for bf16)
                                for kt in range(2):
                                    s_, e_ = kt == 0, kt == 1
                                    nc.tensor.matmul(
                                        rs_ps[:], psT[:, kt, :], onesb_sb[:],
                                        start=s_, stop=e_,
                                    )
                                    for ec in range(2):
                                        nc.tensor.matmul(
                                            o_ps[:, ec * 512 : (ec + 1) * 512],
                                            psT[:, kt, :],
                                            v_bf[:, kt,
                                                 ec * 512 : (ec + 1) * 512],
                                            start=s_, stop=e_,
                                        )
                            else:
                                pslc = psT[:, 2 * half : 2 * half + 2, :]
                                nc.tensor.matmul(
                                    rs_ps[:], pslc, ones_sb[:],
                                    start=first, stop=last, perf_mode=DR,
                                )
                                for ec in range(2):
                                    nc.tensor.matmul(
                                        o_ps[:, ec * 512 : (ec + 1) * 512],
                                        pslc,
                                        v_t[c // 2][
                                            :, 2 * (c % 2) : 2 * (c % 2) + 2,
                                            ec * 512 : (ec + 1) * 512],
                                        start=first, stop=last, perf_mode=DR,
                                    )
                            if last:
                                rcp = spool.tile([128, 1], F32, tag="rcp",
                                                 name="rcp")
                                nc.vector.reciprocal(rcp[:], rs_ps[:])
                                o_sb = opool.tile([128, E], F32, tag="osb",
                                                  name="o_sb")
                                # out = o_ps/(8*rowsum) + bq, in halves so
                                # the first DMA overlaps the second STT
                                for hv in range(2):
                                    sl = slice(hv * 512, (hv + 1) * 512)
                                    nc.vector.scalar_tensor_tensor(
                                        o_sb[:, sl], o_ps[:, sl], rcp[:],
                                        bqb_sb[:, sl], ALU.mult, ALU.add,
                                    )
                                    nc.sync.dma_start(
                                        out[slot * 128 : (slot + 1) * 128, sl],
                                        o_sb[:, sl],
                                    )

                    # PV lags scores by two pairs so the exp latency
                    # never stalls the PE
                    pending = deque()
                    for slot, nch, pairs in _pair_schedule(variant, counts):
                        for pi in range(len(pairs)):
                            psT = emit_scores(slot, pairs, pi)
                            pending.append((slot, nch, pairs, pi, psT))
                            if len(pending) > 3:
                                emit_pv(*pending.popleft())
                    while pending:
                        emit_pv(*pending.popleft())

    return nc


# ---------------------------------------------------------------------------
# Host wrapper.
# ---------------------------------------------------------------------------

_prog_cache = {}


def _get_program(variant):
    if variant not in _prog_cache:
        _prog_cache[variant] = build_program(variant)
    return _prog_cache[variant]


def _analyze_mask(att_mask):
    causal = np.array_equal(
        att_mask, np.triu(np.ones((S, S), dtype=att_mask.dtype), 1)
    )
    if causal:
        return "causal"
    if not att_mask.any():
        return "nomask"
    return "generic"


def _dmajor(x):
    """[s, d] f32 -> contiguous [128, 8, s] fp8 (d split as dt*128+dp)."""
    return _dmajor_raw(x.astype(NP_FP8))


def _dmajor_raw(x8):
    """[s, d] fp8 -> contiguous [128, 8, s]."""
    xT = np.ascontiguousarray(x8.T)  # [d, s]
    return np.ascontiguousarray(xT.reshape(8, 128, -1).transpose(1, 0, 2))


def _causal_mask2(h):
    """[128, 256] fp8 k-major boundary mask (0 keep / -240 drop) for
    half h: [diag|full] for even tiles, [zero|diag] for odd."""
    kk = np.arange(128)[:, None]
    qq = np.arange(128)[None, :]
    diag = np.where(kk > qq, -MASK_VAL, 0.0)
    full = np.full((128, 128), -MASK_VAL)
    zero = np.zeros((128, 128))
    pair = (diag, full) if h == 0 else (zero, diag)
    return np.ascontiguousarray(np.concatenate(pair, axis=1)).astype(NP_FP8)


def _build_in_maps(xq, xk, xv, Wq, bq, att_mask, variant):
    xq = np.asarray(xq, dtype=np.float32)
    xk = np.asarray(xk, dtype=np.float32)
    xv = np.asarray(xv, dtype=np.float32)
    Wq = np.asarray(Wq, dtype=np.float32)
    bq = np.asarray(bq, dtype=np.float32)
    att_mask = np.asarray(att_mask)

    wqT32 = Wq.T * W_SCALE
    wq8_flat = wqT32.astype(NP_FP8)
    wq8 = np.ascontiguousarray(
        wq8_flat.reshape(8, 128, E).transpose(1, 0, 2)
    )
    wqlo8 = np.ascontiguousarray(
        (wqT32 - wq8_flat.astype(np.float32))
        .astype(NP_FP8).reshape(8, 128, E).transpose(1, 0, 2)
    )
    onesb = np.full((128, 1), 32.0, dtype=NP_BF16)
    bq8s = np.ascontiguousarray(bq.reshape(8, 128).T * QKV_SCALE)
    bqb1 = np.ascontiguousarray(np.broadcast_to(bq, (128, E))).astype(
        np.float32
    )
    ones8 = np.full((128, 2, 1), QKV_SCALE, dtype=NP_FP8)
    id240 = (np.eye(128) * MASK_VAL).astype(NP_FP8)
    counts = _chunk_counts(variant)
    tiles_by_half = (TILES_H0, TILES_H1)

    in_maps = []
    for c in range(NCORES):
        b, h = divmod(c, 2)
        tiles = tiles_by_half[h]
        rows = np.concatenate(
            [np.arange(t * 128, (t + 1) * 128) for t in tiles]
        )
        m = {
            "wq8": wq8,
            "xq8": _dmajor(xq[b][rows]),
            "xk8": _dmajor(xk[b]),
            "xv8": _dmajor(xv[b]),
            "bq8s": bq8s,
            "bqb1": bqb1,
            "ones8": ones8,
            "id240": id240,
        }
        if variant == "causal":
            m["mask2"] = _causal_mask2(h)
            m["wqlo8"] = wqlo8
            m["onesb"] = onesb
            xvh = xv[b][:256]  # head keys, same fp8 base as xv8
            rxv = xvh - xvh.astype(NP_FP8).astype(np.float32)
            m["xvlo8"] = _dmajor_raw(rxv.astype(NP_FP8))
        elif variant == "generic":
            mds = []
            for slot, nch, pairs in _pair_schedule(variant, counts):
                t = tiles[slot]
                for a, bch in pairs:
                    md = np.zeros((128, 512))
                    for half, ch in enumerate((a, bch)):
                        if ch is None:
                            continue
                        blk = att_mask[
                            t * 128 : (t + 1) * 128,
                            ch * 256 : (ch + 1) * 256,
                        ]  # [q, k]
                        kmaj = blk.T.astype(np.float64) * -MASK_VAL
                        md[:, half * 256 : half * 256 + 128] = kmaj[:128]
                        md[:, half * 256 + 128 : (half + 1) * 256] = kmaj[128:]
                    mds.append(md.astype(NP_FP8))
            m["maskd"] = np.stack(mds)
        in_maps.append(m)
    return in_maps


def kernel(xq, xk, xv, Wq, bq, att_mask):
    from concourse.bass_utils import run_bass_kernel_spmd

    variant = _analyze_mask(np.asarray(att_mask))
    nc = _get_program(variant)
    in_maps = _build_in_maps(xq, xk, xv, Wq, bq, att_mask, variant)

    res = run_bass_kernel_spmd(nc, in_maps, list(range(NCORES)))

    tiles_by_half = (TILES_H0, TILES_H1)
    out = np.empty((B, S, E), dtype=np.float32)
    for c in range(NCORES):
        b, h = divmod(c, 2)
        tiles = tiles_by_half[h]
        oc = res.results[c]["out"]
        for i, t in enumerate(tiles):
            out[b, t * 128 : (t + 1) * 128, :] = oc[i * 128 : (i + 1) * 128, :]
    return out


# revision 81
# speedup vs baseline: 3.6912x; 1.2043x over previous
"""Single-head attention (shared QKV weight) on 8 Trainium2 NeuronCores.

Problem: B=4, S=2048, D=E=1024
  Q = xq@Wq.T + bq ; K = xk@Wq.T + bq ; V = xv@Wq.T + bq
  out = softmax(mask(Q@K.T/sqrt(E))) @ V

Sharding: data-parallel over batch x query-halves -> 8 cores. Core c
handles batch b=c//2 and half h=c%2 of the query tiles: even global
128-row tiles for h=0, odd for h=1, so slot i on every core processes
exactly i+1 key-chunks of 256 under the causal mask (identical SPMD
instruction stream, balanced FLOPs). Each core computes the full K/V
projection of its batch and its own Q projection.

All matmuls run in fp8e4 (e4m3) with DoubleRow perf mode: two 128-deep
contraction tiles per instruction at 0.5 PE cycles/row -> 4x fp32r
throughput. Scaling keeps everything in fp8 range:
- host supplies x (unit-normal, as-is) and 32*Wq in fp8
- projections evict PSUM/4 -> fp8 (values = 8x true Q/K/V)
- scores PSUM = 64 * QK^T_true; exp applies 1/(64*32) = 1/2048
- attention is k-major: scores land as S^T[k,q], exp writes fp8
  P^T[k,2,q] tiles directly usable as the PV stationary operand (no
  transposes); row-sums come from an N=1 matmul with an 8.0-vector
  (so rs = 8*rowsum matches o_ps = 8*P@V); the final
  out = o_ps*(1/rs) + bq is one scalar_tensor_tensor op.
- the causal boundary mask is applied ON the PE: an extra matmul per
  masked k-tile accumulates 240 * mask8[k,q] (mask8 in {0,-240}) =
  -57600 into the raw-score PSUM (|s| < 5e3), so exp -> 0 with no
  cross-engine hop before the exp.

Only Act and DVE can read PSUM (GPSIMD cannot), so each projection
PSUM group (one [128,2048] tile, 4 banks) is evicted by both lanes in
parallel (two [128,1024] halves), keeping per-group eviction wall time
under the group's matmul time.

Math shortcuts (exact): K-bias cancels in softmax; Q-bias fused into
the Q eviction; V-bias added via the fused final op (softmax rows sum
to 1); scores bounded (|s|/32 <~ 2) so softmax skips max-subtraction.
"""

import re
from collections import deque

import numpy as np
import ml_dtypes

import concourse.bass as bass
import concourse.mybir as mybir
import concourse.tile as tile
from concourse.vector_clock import ScopedClock

F32 = mybir.dt.float32
FP8 = mybir.dt.float8e4
BF16 = mybir.dt.bfloat16
NP_FP8 = ml_dtypes.float8_e4m3
NP_BF16 = ml_dtypes.bfloat16
AF = mybir.ActivationFunctionType
ALU = mybir.AluOpType
DR = mybir.MatmulPerfMode.DoubleRow

B, S, D, E = 4, 2048, 1024, 1024
NCORES = 8
W_SCALE = 32.0  # host multiplies Wq by this before fp8 conversion
QKV_SCALE = 8.0  # fp8 Q/K/V values are 8x the true values
EVICT_SCALE = QKV_SCALE / W_SCALE  # PSUM -> fp8 eviction multiplier
EXP_SCALE = (1.0 / 32.0) / (QKV_SCALE * QKV_SCALE)  # softmax scale / 64
MASK_VAL = 240.0  # ident240 @ mask8(-240) adds -57600 to raw scores

# Half h owns global q-tiles h, h+2, ..., h+14. Slot i (its i-th local
# 128-row tile) is global tile 2i+h and needs ceil((2i+h+1)/2) = i+1
# chunks of 256 keys -> identical instruction stream on all cores.
TILES_H0 = [0, 2, 4, 6, 8, 10, 12, 14]
TILES_H1 = [1, 3, 5, 7, 9, 11, 13, 15]

# ---------------------------------------------------------------------------
# Workarounds for this container's walrus build, which rejects any
# instruction carrying more than one semaphore wait.
# ---------------------------------------------------------------------------

_split_counter = [0]


def _legalize_waits(nc):
    """Move all-but-one sem wait from each instruction onto single-wait
    NoOps inserted immediately before it on the same engine. Engines
    dispatch in order, so the nops' waits are satisfied before the
    instruction issues."""
    for f in nc.m.functions:
        for bb in f.blocks:
            insts = list(bb.instructions)
            out = []
            changed = False
            for inst in insts:
                si = inst.sync_info
                if si is not None and si.on_wait is not None and len(si.on_wait) > 1:
                    waits = list(si.on_wait)
                    for w in waits[:-1]:
                        _split_counter[0] += 1
                        nop = mybir.InstNoOp(
                            name=f"I-waitsplit-{_split_counter[0]}",
                            opcode="NoOp",
                            engine=inst.engine,
                            sync_info=mybir.SyncInfo(on_wait=[w], on_update=[]),
                        )
                        nc.register_instruction(nop)
                        out.append(nop)
                    si.on_wait = [waits[-1]]
                    changed = True
                out.append(inst)
            if changed:
                bb.instructions = out


class _TileContext(tile.TileContext):
    def __init__(self, nc, **kw):
        kw.setdefault("pool_alloc_mode", "queue")
        super().__init__(nc, **kw)

    def _drain_and_barrier(self, tick_clock, wait_clock):
        gc = tick_clock.global_clock
        m = re.search(r"\[([0-9, ]*)\]", repr(gc))
        ticks = (
            [int(x) for x in m.group(1).split(",")]
            if m and m.group(1).strip()
            else []
        )
        for p, t in [(i, t) for i, t in enumerate(ticks) if t > 0]:
            nop = self.nc.sync.nop(nofuse=True, hint="drain_split")
            sc = ScopedClock({})
            sc.require_at_least(None, p, t)
            wait_clock.add_sem_waits(nop.ins, sc)
        self.nc.sync.drain()
        self.nc.all_engine_barrier()
        assert self.sems is not None
        popped = self.nc._tile_sem_poison_stack.pop()
        assert popped is self._sem_poison
        self.nc.clear_and_free_semaphores(list(self.sems.allocated().values()))
        self.nc.all_engine_barrier()

    def __exit__(self, *args):
        r = super().__exit__(*args)
        _legalize_waits(self.nc)
        return r


# ---------------------------------------------------------------------------
# Device program (identical on all 8 cores).
# ---------------------------------------------------------------------------


def _chunk_counts(variant):
    return [1, 2, 3, 4, 5, 6, 7, 8] if variant == "causal" else [8] * 8


def _pair_schedule(variant, counts):
    """Per slot: list of chunk-pairs (a, b) (b may be None). The masked
    chunk (causal) is ordered first; slots run in ascending size so the
    tail after the last slot's PE work is only one final chain."""
    slots = []
    for slot in range(8):
        nch = counts[slot]
        if variant == "causal":
            order = [nch - 1] + list(range(nch - 1))
        else:
            order = list(range(nch))
        pairs = [
            (order[i], order[i + 1] if i + 1 < nch else None)
            for i in range(0, nch, 2)
        ]
        slots.append((slot, nch, pairs))
    return slots


def build_program(variant, repeat=1):
    """variant: 'causal' (slot i gets i+1 key-chunks of 256, one shared
    boundary mask), 'nomask' (8 chunks, no masks), 'generic' (8 chunks,
    per-chunk-pair masks streamed from DRAM). repeat: run the whole
    body N times (timing aid; output identical)."""
    counts = _chunk_counts(variant)
    npairs_tot = sum((c + 1) // 2 for c in counts)

    nc = bass.Bass("TRN2", target_bir_lowering=False, debug=False)
    wq8 = nc.declare_dram_parameter("wq8", [128, 8, E], FP8, isOutput=False)
    xq8 = nc.declare_dram_parameter("xq8", [128, 8, 1024], FP8, isOutput=False)
    xk8 = nc.declare_dram_parameter("xk8", [128, 8, S], FP8, isOutput=False)
    xv8 = nc.declare_dram_parameter("xv8", [128, 8, S], FP8, isOutput=False)
    bq8s = nc.declare_dram_parameter("bq8s", [128, 8], F32, isOutput=False)
    bqb1 = nc.declare_dram_parameter("bqb1", [128, E], F32, isOutput=False)
    ones8 = nc.declare_dram_parameter("ones8", [128, 2, 1], FP8, isOutput=False)
    id240 = nc.declare_dram_parameter("id240", [128, 128], FP8, isOutput=False)
    if variant == "causal":
        # fp8 residuals of xv-head / W for the high-precision V head
        # (output rows 0..255 see V almost unaveraged, so slot 0 uses a
        # residual-corrected bf16 V and bf16 P)
        wqlo8 = nc.declare_dram_parameter("wqlo8", [128, 8, E], FP8, isOutput=False)
        xvlo8 = nc.declare_dram_parameter("xvlo8", [128, 8, 256], FP8, isOutput=False)
        onesb = nc.declare_dram_parameter("onesb", [128, 1], BF16, isOutput=False)
        mask2 = nc.declare_dram_parameter("mask2", [128, 256], FP8, isOutput=False)
    elif variant == "generic":
        maskd = nc.declare_dram_parameter(
            "maskd", [npairs_tot, 128, 512], FP8, isOutput=False
        )
    out = nc.declare_dram_parameter("out", [1024, E], F32, isOutput=True)

    with _TileContext(nc) as tc:
        with (
            tc.tile_pool(name="const", bufs=1) as cpool,
            tc.tile_pool(name="big", bufs=1) as bpool,
        ):
            for _rep in range(repeat):
                wq_ctx = tc.tile_pool(name=f"wqpool{_rep}", bufs=1)
                wqpool = wq_ctx.__enter__()
                # Inputs land as chunk tiles in first-use order (the cost
                # model serializes transfers on one DMA-engine pool), so
                # each projection chunk's input precedes its matmuls.
                def _xt(src, t, c0, nch):
                    x = bpool.tile([128, 8, 512 * nch], FP8, tag=t, name="x")
                    nc.sync.dma_start(
                        x[:], src[:, :, c0 * 512 : (c0 + nch) * 512]
                    )
                    return x

                # DMA sizes graded so each projection chunk's input lands
                # just before the PE reaches it on the serialized DMA pool
                wq_t = [None] * 4

                def _wq(dtp):
                    w = wqpool.tile([128, 2, E], FP8, tag=f"wq{dtp}", name="w")
                    nc.sync.dma_start(w[:], wq8[:, 2 * dtp : 2 * dtp + 2, :])
                    wq_t[dtp] = w

                def wq_slc(dtp, lo, hi):
                    return wq_t[dtp][:, :, lo:hi]

                _wq(0)
                xk_t0 = _xt(xk8, "xk0", 0, 1)
                _wq(1)
                _wq(2)
                _wq(3)
                xk_t1 = _xt(xk8, "xk1", 1, 1)
                xk_t2 = _xt(xk8, "xk2", 2, 2)
                xq_sb = _xt(xq8, "xq", 0, 2)
                if _rep == 0:
                    bq8_sb = cpool.tile([128, 8], F32, tag="bq8")
                    nc.sync.dma_start(bq8_sb[:], bq8s[:])
                xv_t0 = _xt(xv8, "xv0", 0, 2)
                xv_t1 = _xt(xv8, "xv1", 2, 2)

                def xk_slc(ch, dtp):
                    if ch < 2:
                        t, off = (xk_t0, xk_t1)[ch], 0
                    else:
                        t, off = xk_t2, (ch - 2) * 512
                    return t[:, 2 * dtp : 2 * dtp + 2, off : off + 512]

                def xq_slc(ch, dtp):
                    return xq_sb[
                        :, 2 * dtp : 2 * dtp + 2, ch * 512 : (ch + 1) * 512
                    ]

                def xv_slc(sb4, dtp, lo, hi):
                    t = (xv_t0, xv_t1)[sb4 // 2]
                    off = (sb4 % 2) * 512
                    return t[:, 2 * dtp : 2 * dtp + 2, off + lo : off + hi]
                if variant == "causal":
                    wqlo_sb = wqpool.tile([128, 8, E], FP8, tag="wqlo",
                                          name="wqlo_sb")
                    nc.sync.dma_start(wqlo_sb[:], wqlo8[:])
                    xvlo_sb = wqpool.tile([128, 8, 256], FP8, tag="xvlo",
                                          name="xvlo_sb")
                    nc.sync.dma_start(xvlo_sb[:], xvlo8[:])
                if _rep == 0:
                    bqb_sb = cpool.tile([128, E], F32, tag="bqb")
                    nc.sync.dma_start(bqb_sb[:], bqb1[:])
                    ones_sb = cpool.tile([128, 2, 1], FP8, tag="ones")
                    nc.sync.dma_start(ones_sb[:], ones8[:])
                    id_sb = cpool.tile([128, 128], FP8, tag="id240")
                    nc.sync.dma_start(id_sb[:], id240[:])
                    if variant == "causal":
                        onesb_sb = cpool.tile([128, 1], BF16, tag="onesb")
                        nc.sync.dma_start(onesb_sb[:], onesb[:])
                        mask_sb = cpool.tile([128, 256], FP8, tag="mask2")
                        nc.sync.dma_start(mask_sb[:], mask2[:])

                q_sb = bpool.tile([128, 8, 1024], FP8, tag="q")
                k_sb = bpool.tile([128, 8, S], FP8, tag="k")
                v_t = [
                    bpool.tile([128, 4, E], FP8, tag=f"v{i}", name="v")
                    for i in range(4)
                ]
                if variant == "causal":
                    v_bf = bpool.tile([128, 2, E], BF16, tag="vbf")

                # ---- projections ----
                # Each PSUM group is one [128,2048] tile (4 banks, two
                # groups in flight). Both PSUM-capable lanes (Act, DVE;
                # GPSIMD cannot read PSUM) evict each group in parallel
                # halves, so eviction wall time (~1.3us) stays under the
                # group's matmul time (~1.7us).
                with tc.tile_pool(name=f"projps{_rep}", bufs=2, space="PSUM") as ppsum:
                    # K^T then Q^T (e-major): out[e,s] += (wq8[d,e]).T@x[d,s]
                    for x_slc, dst, nch, with_bias in (
                        (xk_slc, k_sb, 4, False),
                        (xq_slc, q_sb, 2, True),
                    ):
                        for ch in range(nch):
                            for g in range(2):
                                pp4 = [
                                    ppsum.tile([128, 512], F32,
                                               tag=f"pp{i}", name="pp4")
                                    for i in range(4)
                                ]
                                for dtp in range(4):
                                    xslc = x_slc(ch, dtp)
                                    for ei in range(4):
                                        et = g * 4 + ei
                                        nc.tensor.matmul(
                                            pp4[ei][:],
                                            wq_p[dtp][
                                                :, :, et * 128 : (et + 1) * 128
                                            ],
                                            xslc,
                                            start=(dtp == 0),
                                            stop=(dtp == 3),
                                            perf_mode=DR,
                                        )
                                if with_bias:
                                    # Act: ei 0,1 (activation bias);
                                    # DVE: ei 2,3 (tensor_scalar w/ bias AP)
                                    for ei in range(4):
                                        et = g * 4 + ei
                                        dslc = dst[
                                            :, et, ch * 512 : (ch + 1) * 512
                                        ]
                                        pslc = pp4[ei][:]
                                        if ei >= 2:
                                            nc.scalar.activation(
                                                dslc, pslc, AF.Identity,
                                                bias=bq8_sb[:, et : et + 1],
                                                scale=EVICT_SCALE,
                                            )
                                        else:
                                            nc.vector.tensor_scalar(
                                                dslc, pslc, EVICT_SCALE,
                                                bq8_sb[:, et : et + 1],
                                                ALU.mult, ALU.add,
                                            )
                                else:
                                    for ei in range(4):
                                        et = g * 4 + ei
                                        d_ = dst[
                                            :, et, ch * 512 : (ch + 1) * 512
                                        ]
                                        if ei % 2 == 0:
                                            nc.vector.tensor_scalar_mul(
                                                d_, pp4[ei][:], EVICT_SCALE,
                                            )
                                        else:
                                            nc.scalar.activation(
                                                d_, pp4[ei][:],
                                                AF.Identity,
                                                scale=EVICT_SCALE,
                                            )

                    # V (s-major): out[s, e] += x[d, s].T @ wq8[d, e]
                    for sb4 in range(4):
                        for g in range(2):
                            pp4 = [
                                ppsum.tile([128, 512], F32,
                                           tag=f"pp{i}", name="pp4")
                                for i in range(4)
                            ]
                            head = variant == "causal" and sb4 == 0 and g == 0
                            for dtp in range(4):
                                for s2 in range(2):
                                    si = g * 2 + s2
                                    for ec in range(2):
                                        nc.tensor.matmul(
                                            pp4[s2 * 2 + ec][:],
                                            xv_slc(
                                                sb4, dtp,
                                                si * 128, (si + 1) * 128,
                                            ),
                                            wq_p[dtp][
                                                :, :, ec * 512 : (ec + 1) * 512
                                            ],
                                            start=(dtp == 0),
                                            stop=(dtp == 3) and not head,
                                            perf_mode=DR,
                                        )
                            if head:
                                # residual passes accumulate rx@W8 + x8@rW
                                # into the same PSUM: V head reaches ~bf16
                                # accuracy (residuals are small, their fp8
                                # quantization error is ~2^-10 absolute)
                                for dtp in range(4):
                                    for si in range(2):
                                        for ec in range(2):
                                            cslc = pp4[si * 2 + ec][:]
                                            nc.tensor.matmul(
                                                cslc,
                                                xvlo_sb[
                                                    :, 2 * dtp : 2 * dtp + 2,
                                                    si * 128 : (si + 1) * 128,
                                                ],
                                                wq_p[dtp][
                                                    :, :,
                                                    ec * 512 : (ec + 1) * 512,
                                                ],
                                                start=False, stop=False,
                                                perf_mode=DR,
                                            )
                                            nc.tensor.matmul(
                                                cslc,
                                                xv_slc(
                                                    0, dtp,
                                                    si * 128, (si + 1) * 128,
                                                ),
                                                wqlo_sb[
                                                    :, 2 * dtp : 2 * dtp + 2,
                                                    ec * 512 : (ec + 1) * 512,
                                                ],
                                                start=False,
                                                stop=(dtp == 3),
                                                perf_mode=DR,
                                            )
                                nc.scalar.activation(
                                    v_bf[:, 0, 0:512], pp4[0][:], AF.Copy,
                                )
                                nc.scalar.activation(
                                    v_bf[:, 0, 512:1024], pp4[1][:], AF.Copy,
                                )
                                nc.vector.tensor_copy(
                                    v_bf[:, 1, 0:512], pp4[2][:],
                                )
                                nc.vector.tensor_copy(
                                    v_bf[:, 1, 512:1024], pp4[3][:],
                                )
                            for ei in range(4):
                                d_ = v_t[sb4][
                                    :, g * 2 + ei // 2,
                                    (ei % 2) * 512 : (ei % 2 + 1) * 512,
                                ]
                                if ei % 2 == 0:
                                    nc.scalar.activation(
                                        d_, pp4[ei][:],
                                        AF.Identity, scale=EVICT_SCALE,
                                    )
                                else:
                                    nc.vector.tensor_scalar_mul(
                                        d_, pp4[ei][:], EVICT_SCALE,
                                    )

                # ---- attention (k-major, chunk pairs) ----
                wq_ctx.__exit__(None, None, None)
                with (
                    tc.tile_pool(name=f"pt{_rep}", bufs=4) as ptpool,
                    tc.tile_pool(name=f"small{_rep}", bufs=4) as spool,
                    tc.tile_pool(name=f"mstage{_rep}", bufs=2) as mpool,
                    tc.tile_pool(name=f"opool{_rep}", bufs=2) as opool,
                    tc.tile_pool(name=f"sps{_rep}", bufs=3, space="PSUM") as spsum,
                    tc.tile_pool(name=f"rps{_rep}", bufs=1, space="PSUM") as rpsum,
                    tc.tile_pool(name=f"ops{_rep}", bufs=2, space="PSUM") as opsum,
                ):
                    slot_ps = {}
                    pair_i = [0]

                    def emit_scores(slot, pairs, pi):
                        a, b = pairs[pi]
                        if variant == "causal":
                            masked_cols = [0, 128] if pi == 0 else []
                            msk = mask_sb
                        elif variant == "generic":
                            masked_cols = [0, 128] + ([256, 384] if b is not None else [])
                            msk = mpool.tile([128, 512], FP8, tag="msk", name="m")
                            nc.sync.dma_start(msk[:], maskd[pair_i[0]])
                        else:
                            masked_cols = []
                        pair_i[0] += 1
                        s_ps = spsum.tile([128, 512], F32, tag="s", name="s_ps")
                        for half, c in enumerate((a, b)):
                            if c is None:
                                continue
                            for kt in range(2):
                                kk = 2 * c + kt
                                col = half * 256 + kt * 128
                                has_mask = col in masked_cols
                                for etp in range(4):
                                    nc.tensor.matmul(
                                        s_ps[:, col : col + 128],
                                        k_sb[:, 2 * etp : 2 * etp + 2,
                                             kk * 128 : (kk + 1) * 128],
                                        q_sb[:, 2 * etp : 2 * etp + 2,
                                             slot * 128 : (slot + 1) * 128],
                                        start=(etp == 0),
                                        stop=(etp == 3) and not has_mask,
                                        perf_mode=DR,
                                    )
                                if has_mask:
                                    # accumulate 240*mask8 (= -57600 on
                                    # masked positions) on the PE itself
                                    nc.tensor.matmul(
                                        s_ps[:, col : col + 128],
                                        id_sb[:],
                                        msk[:, col : col + 128],
                                        start=False, stop=True,
                                    )
                        wid = 512 if b is not None else 256
                        if variant == "causal" and slot == 0:
                            # short output rows: bf16 P (and bf16 V) for
                            # the only chunk slot 0 attends
                            psT = ptpool.tile([128, 2, 128], BF16,
                                              tag="ptbf", name="psTb")
                            nc.scalar.activation(
                                psT[:], s_ps[:, 0:256], AF.Exp,
                                scale=EXP_SCALE,
                            )
                        else:
                            psT = ptpool.tile([128, 4, 128], FP8, tag="pt",
                                              name="psT")
                            nc.scalar.activation(
                                psT[:, 0 : wid // 128, :], s_ps[:, 0:wid],
                                AF.Exp, scale=EXP_SCALE,
                            )
                        return psT

                    def emit_pv(slot, nch, pairs, pi, psT):
                        for half, c in enumerate(pairs[pi]):
                            if c is None:
                                continue
                            pos = 2 * pi + half
                            first = pos == 0
                            last = pos == nch - 1
                            if first:
                                o_t = opsum.tile(
                                    [128, 1024], F32, tag="o", name="o_t"
                                )
                                rs_t = rpsum.tile(
                                    [128, 1], F32, tag="rs", name="rs_t"
                                )
                                slot_ps[slot] = (o_t, rs_t)
                            o_ps, rs_ps = slot_ps[slot]
                            if variant == "causal" and slot == 0:
                                # bf16 PV over the residual-corrected V
                                # head (no DoubleRow for bf16)
                                for kt in range(2):
                                    s_, e_ = kt == 0, kt == 1
                                    nc.tensor.matmul(
                                        rs_ps[:], psT[:, kt, :], onesb_sb[:],
                                        start=s_, stop=e_,
                                    )
                                    for ec in range(2):
                                        nc.tensor.matmul(
                                            o_ps[:, ec * 512 : (ec + 1) * 512],
                                            psT[:, kt, :],
                                            v_bf[:, kt,
                                                 ec * 512 : (ec + 1) * 512],
                                            start=s_, stop=e_,
                                        )
                            else:
                                pslc = psT[:, 2 * half : 2 * half + 2, :]
                                nc.tensor.matmul(
                                    rs_ps[:], pslc, ones_sb[:],
                                    start=first, stop=last, perf_mode=DR,
                                )
                                for ec in range(2):
                                    nc.tensor.matmul(
                                        o_ps[:, ec * 512 : (ec + 1) * 512],
                                        pslc,
                                        v_t[c // 2][
                                            :, 2 * (c % 2) : 2 * (c % 2) + 2,
                                            ec * 512 : (ec + 1) * 512],
                                        start=first, stop=last, perf_mode=DR,
                                    )
                            if last:
                                rcp = spool.tile([128, 1], F32, tag="rcp",
                                                 name="rcp")
                                nc.vector.reciprocal(rcp[:], rs_ps[:])
                                o_sb = opool.tile([128, E], F32, tag="osb",
                                                  name="o_sb")
                                # out = o_ps/(8*rowsum) + bq, in halves so
                                # the first DMA overlaps the second STT
                                for hv in range(2):
                                    sl = slice(hv * 512, (hv + 1) * 512)
                                    nc.vector.scalar_tensor_tensor(
                                        o_sb[:, sl], o_ps[:, sl], rcp[:],
                                        bqb_sb[:, sl], ALU.mult, ALU.add,
                                    )
                                    nc.sync.dma_start(
                                        out[slot * 128 : (slot + 1) * 128, sl],
                                        o_sb[:, sl],
                                    )

                    # PV lags scores by two pairs so the exp latency
                    # never stalls the PE
                    pending = deque()
                    for slot, nch, pairs in _pair_schedule(variant, counts):
                        for pi in range(len(pairs)):
                            psT = emit_scores(slot, pairs, pi)
                            pending.append((slot, nch, pairs, pi, psT))
                            if len(pending) > 2:
                                emit_pv(*pending.popleft())
                    while pending:
                        emit_pv(*pending.popleft())

    return nc


# ---------------------------------------------------------------------------
# Host wrapper.
# ---------------------------------------------------------------------------

_prog_cache = {}


def _get_program(variant):
    if variant not in _prog_cache:
        _prog_cache[variant] = build_program(variant)
    return _prog_cache[variant]


def _analyze_mask(att_mask):
    causal = np.array_equal(
        att_mask, np.triu(np.ones((S, S), dtype=att_mask.dtype), 1)
    )
    if causal:
        return "causal"
    if not att_mask.any():
        return "nomask"
    return "generic"


def _dmajor(x):
    """[s, d] f32 -> contiguous [128, 8, s] fp8 (d split as dt*128+dp)."""
    return _dmajor_raw(x.astype(NP_FP8))


def _dmajor_raw(x8):
    """[s, d] fp8 -> contiguous [128, 8, s]."""
    xT = np.ascontiguousarray(x8.T)  # [d, s]
    return np.ascontiguousarray(xT.reshape(8, 128, -1).transpose(1, 0, 2))


def _causal_mask2(h):
    """[128, 256] fp8 k-major boundary mask (0 keep / -240 drop) for
    half h: [diag|full] for even tiles, [zero|diag] for odd."""
    kk = np.arange(128)[:, None]
    qq = np.arange(128)[None, :]
    diag = np.where(kk > qq, -MASK_VAL, 0.0)
    full = np.full((128, 128), -MASK_VAL)
    zero = np.zeros((128, 128))
    pair = (diag, full) if h == 0 else (zero, diag)
    return np.ascontiguousarray(np.concatenate(pair, axis=1)).astype(NP_FP8)


def _build_in_maps(xq, xk, xv, Wq, bq, att_mask, variant):
    xq = np.asarray(xq, dtype=np.float32)
    xk = np.asarray(xk, dtype=np.float32)
    xv = np.asarray(xv, dtype=np.float32)
    Wq = np.asarray(Wq, dtype=np.float32)
    bq = np.asarray(bq, dtype=np.float32)
    att_mask = np.asarray(att_mask)

    wqT32 = Wq.T * W_SCALE
    wq8_flat = wqT32.astype(NP_FP8)
    wq8 = np.ascontiguousarray(
        wq8_flat.reshape(8, 128, E).transpose(1, 0, 2)
    )
    wqlo8 = np.ascontiguousarray(
        (wqT32 - wq8_flat.astype(np.float32))
        .astype(NP_FP8).reshape(8, 128, E).transpose(1, 0, 2)
    )
    onesb = np.full((128, 1), 32.0, dtype=NP_BF16)
    bq8s = np.ascontiguousarray(bq.reshape(8, 128).T * QKV_SCALE)
    bqb1 = np.ascontiguousarray(np.broadcast_to(bq, (128, E))).astype(
        np.float32
    )
    ones8 = np.full((128, 2, 1), QKV_SCALE, dtype=NP_FP8)
    id240 = (np.eye(128) * MASK_VAL).astype(NP_FP8)
    counts = _chunk_counts(variant)
    tiles_by_half = (TILES_H0, TILES_H1)

    in_maps = []
    for c in range(NCORES):
        b, h = divmod(c, 2)
        tiles = tiles_by_half[h]
        rows = np.concatenate(
            [np.arange(t * 128, (t + 1) * 128) for t in tiles]
        )
        m = {
            "wq8": wq8,
            "xq8": _dmajor(xq[b][rows]),
            "xk8": _dmajor(xk[b]),
            "xv8": _dmajor(xv[b]),
            "bq8s": bq8s,
            "bqb1": bqb1,
            "ones8": ones8,
            "id240": id240,
        }
        if variant == "causal":
            m["mask2"] = _causal_mask2(h)
            m["wqlo8"] = wqlo8
            m["onesb"] = onesb
            xvh = xv[b][:256]  # head keys, same fp8 base as xv8
            rxv = xvh - xvh.astype(NP_FP8).astype(np.float32)
            m["xvlo8"] = _dmajor_raw(rxv.astype(NP_FP8))
        elif variant == "generic":
            mds = []
            for slot, nch, pairs in _pair_schedule(variant, counts):
                t = tiles[slot]
                for a, bch in pairs:
                    md = np.zeros((128, 512))
                    for half, ch in enumerate((a, bch)):
                        if ch is None:
                            continue
                        blk = att_mask[
                            t * 128 : (t + 1) * 128,
                            ch * 256 : (ch + 1) * 256,
                        ]  # [q, k]
                        kmaj = blk.T.astype(np.float64) * -MASK_VAL
                        md[:, half * 256 : half * 256 + 128] = kmaj[:128]
                        md[:, half * 256 + 128 : (half + 1) * 256] = kmaj[128:]
                    mds.append(md.astype(NP_FP8))
            m["maskd"] = np.stack(mds)
        in_maps.append(m)
    return in_maps


def kernel(xq, xk, xv, Wq, bq, att_mask):
    from concourse.bass_utils import run_bass_kernel_spmd

    variant = _analyze_mask(np.asarray(att_mask))
    nc = _get_program(variant)
    in_maps = _build_in_maps(xq, xk, xv, Wq, bq, att_mask, variant)

    res = run_bass_kernel_spmd(nc, in_maps, list(range(NCORES)))

    tiles_by_half = (TILES_H0, TILES_H1)
    out = np.empty((B, S, E), dtype=np.float32)
    for c in range(NCORES):
        b, h = divmod(c, 2)
        tiles = tiles_by_half[h]
        oc = res.results[c]["out"]
        for i, t in enumerate(tiles):
            out[b, t * 128 : (t + 1) * 128, :] = oc[i * 128 : (i + 1) * 128, :]
    return out


# revision 86
# speedup vs baseline: 3.7801x; 1.0241x over previous
"""Single-head attention (shared QKV weight) on 8 Trainium2 NeuronCores.

Problem: B=4, S=2048, D=E=1024
  Q = xq@Wq.T + bq ; K = xk@Wq.T + bq ; V = xv@Wq.T + bq
  out = softmax(mask(Q@K.T/sqrt(E))) @ V

Sharding: data-parallel over batch x query-halves -> 8 cores. Core c
handles batch b=c//2 and half h=c%2 of the query tiles: even global
128-row tiles for h=0, odd for h=1, so slot i on every core processes
exactly i+1 key-chunks of 256 under the causal mask (identical SPMD
instruction stream, balanced FLOPs). Each core computes the full K/V
projection of its batch and its own Q projection.

All matmuls run in fp8e4 (e4m3) with DoubleRow perf mode: two 128-deep
contraction tiles per instruction at 0.5 PE cycles/row -> 4x fp32r
throughput. Scaling keeps everything in fp8 range:
- host supplies x (unit-normal, as-is) and 32*Wq in fp8
- projections evict PSUM/4 -> fp8 (values = 8x true Q/K/V)
- scores PSUM = 64 * QK^T_true; exp applies 1/(64*32) = 1/2048
- attention is k-major: scores land as S^T[k,q], exp writes fp8
  P^T[k,2,q] tiles directly usable as the PV stationary operand (no
  transposes); row-sums come from an N=1 matmul with an 8.0-vector
  (so rs = 8*rowsum matches o_ps = 8*P@V); the final
  out = o_ps*(1/rs) + bq is one scalar_tensor_tensor op.
- the causal boundary mask is applied ON the PE: an extra matmul per
  masked k-tile accumulates 240 * mask8[k,q] (mask8 in {0,-240}) =
  -57600 into the raw-score PSUM (|s| < 5e3), so exp -> 0 with no
  cross-engine hop before the exp.

Only Act and DVE can read PSUM (GPSIMD cannot), so each projection
PSUM group (one [128,2048] tile, 4 banks) is evicted by both lanes in
parallel (two [128,1024] halves), keeping per-group eviction wall time
under the group's matmul time.

Math shortcuts (exact): K-bias cancels in softmax; Q-bias fused into
the Q eviction; V-bias added via the fused final op (softmax rows sum
to 1); scores bounded (|s|/32 <~ 2) so softmax skips max-subtraction.
"""

import re
from collections import deque

import numpy as np
import ml_dtypes

import concourse.bass as bass
import concourse.mybir as mybir
import concourse.tile as tile
from concourse.vector_clock import ScopedClock

F32 = mybir.dt.float32
FP8 = mybir.dt.float8e4
BF16 = mybir.dt.bfloat16
NP_FP8 = ml_dtypes.float8_e4m3
NP_BF16 = ml_dtypes.bfloat16
AF = mybir.ActivationFunctionType
ALU = mybir.AluOpType
DR = mybir.MatmulPerfMode.DoubleRow

B, S, D, E = 4, 2048, 1024, 1024
NCORES = 8
W_SCALE = 32.0  # host multiplies Wq by this before fp8 conversion
QKV_SCALE = 8.0  # fp8 Q/K/V values are 8x the true values
EVICT_SCALE = QKV_SCALE / W_SCALE  # PSUM -> fp8 eviction multiplier
EXP_SCALE = (1.0 / 32.0) / (QKV_SCALE * QKV_SCALE)  # softmax scale / 64
MASK_VAL = 240.0  # ident240 @ mask8(-240) adds -57600 to raw scores

# Half h owns global q-tiles h, h+2, ..., h+14. Slot i (its i-th local
# 128-row tile) is global tile 2i+h and needs ceil((2i+h+1)/2) = i+1
# chunks of 256 keys -> identical instruction stream on all cores.
TILES_H0 = [0, 2, 4, 6, 8, 10, 12, 14]
TILES_H1 = [1, 3, 5, 7, 9, 11, 13, 15]

# ---------------------------------------------------------------------------
# Workarounds for this container's walrus build, which rejects any
# instruction carrying more than one semaphore wait.
# ---------------------------------------------------------------------------

_split_counter = [0]


def _legalize_waits(nc):
    """Move all-but-one sem wait from each instruction onto single-wait
    NoOps inserted immediately before it on the same engine. Engines
    dispatch in order, so the nops' waits are satisfied before the
    instruction issues."""
    for f in nc.m.functions:
        for bb in f.blocks:
            insts = list(bb.instructions)
            out = []
            changed = False
            for inst in insts:
                si = inst.sync_info
                if si is not None and si.on_wait is not None and len(si.on_wait) > 1:
                    waits = list(si.on_wait)
                    for w in waits[:-1]:
                        _split_counter[0] += 1
                        nop = mybir.InstNoOp(
                            name=f"I-waitsplit-{_split_counter[0]}",
                            opcode="NoOp",
                            engine=inst.engine,
                            sync_info=mybir.SyncInfo(on_wait=[w], on_update=[]),
                        )
                        nc.register_instruction(nop)
                        out.append(nop)
                    si.on_wait = [waits[-1]]
                    changed = True
                out.append(inst)
            if changed:
                bb.instructions = out


class _TileContext(tile.TileContext):
    def __init__(self, nc, **kw):
        kw.setdefault("pool_alloc_mode", "queue")
        super().__init__(nc, **kw)

    def _drain_and_barrier(self, tick_clock, wait_clock):
        gc = tick_clock.global_clock
        m = re.search(r"\[([0-9, ]*)\]", repr(gc))
        ticks = (
            [int(x) for x in m.group(1).split(",")]
            if m and m.group(1).strip()
            else []
        )
        for p, t in [(i, t) for i, t in enumerate(ticks) if t > 0]:
            nop = self.nc.sync.nop(nofuse=True, hint="drain_split")
            sc = ScopedClock({})
            sc.require_at_least(None, p, t)
            wait_clock.add_sem_waits(nop.ins, sc)
        self.nc.sync.drain()
        self.nc.all_engine_barrier()
        assert self.sems is not None
        popped = self.nc._tile_sem_poison_stack.pop()
        assert popped is self._sem_poison
        self.nc.clear_and_free_semaphores(list(self.sems.allocated().values()))
        self.nc.all_engine_barrier()

    def __exit__(self, *args):
        r = super().__exit__(*args)
        _legalize_waits(self.nc)
        return r


# ---------------------------------------------------------------------------
# Device program (identical on all 8 cores).
# ---------------------------------------------------------------------------


def _chunk_counts(variant):
    return [1, 2, 3, 4, 5, 6, 7, 8] if variant == "causal" else [8] * 8


def _pair_schedule(variant, counts):
    """Per slot: list of chunk-pairs (a, b) (b may be None). The masked
    chunk (causal) is ordered first; slots run in ascending size so the
    tail after the last slot's PE work is only one final chain."""
    slots = []
    for slot in range(8):
        nch = counts[slot]
        if variant == "causal":
            order = [nch - 1] + list(range(nch - 1))
        else:
            order = list(range(nch))
        pairs = [
            (order[i], order[i + 1] if i + 1 < nch else None)
            for i in range(0, nch, 2)
        ]
        slots.append((slot, nch, pairs))
    return slots


def build_program(variant, repeat=1):
    """variant: 'causal' (slot i gets i+1 key-chunks of 256, one shared
    boundary mask), 'nomask' (8 chunks, no masks), 'generic' (8 chunks,
    per-chunk-pair masks streamed from DRAM). repeat: run the whole
    body N times (timing aid; output identical)."""
    counts = _chunk_counts(variant)
    npairs_tot = sum((c + 1) // 2 for c in counts)

    nc = bass.Bass("TRN2", target_bir_lowering=False, debug=False)
    wq8 = nc.declare_dram_parameter("wq8", [128, 8, E], FP8, isOutput=False)
    xq8 = nc.declare_dram_parameter("xq8", [128, 8, 1024], FP8, isOutput=False)
    xk8 = nc.declare_dram_parameter("xk8", [128, 8, S], FP8, isOutput=False)
    xv8 = nc.declare_dram_parameter("xv8", [128, 8, S], FP8, isOutput=False)
    bq8s = nc.declare_dram_parameter("bq8s", [128, 8], F32, isOutput=False)
    bqb1 = nc.declare_dram_parameter("bqb1", [128, E], F32, isOutput=False)
    ones8 = nc.declare_dram_parameter("ones8", [128, 2, 1], FP8, isOutput=False)
    id240 = nc.declare_dram_parameter("id240", [128, 128], FP8, isOutput=False)
    if variant == "causal":
        # fp8 residuals of xv-head / W for the high-precision V head
        # (output rows 0..255 see V almost unaveraged, so slot 0 uses a
        # residual-corrected bf16 V and bf16 P)
        wqlo8 = nc.declare_dram_parameter("wqlo8", [128, 8, E], FP8, isOutput=False)
        xvlo8 = nc.declare_dram_parameter("xvlo8", [128, 8, 128], FP8, isOutput=False)
        onesb = nc.declare_dram_parameter("onesb", [128, 1], BF16, isOutput=False)
        mask2 = nc.declare_dram_parameter("mask2", [128, 256], FP8, isOutput=False)
    elif variant == "generic":
        maskd = nc.declare_dram_parameter(
            "maskd", [npairs_tot, 128, 512], FP8, isOutput=False
        )
    out = nc.declare_dram_parameter("out", [1024, E], F32, isOutput=True)

    with _TileContext(nc) as tc:
        with (
            tc.tile_pool(name="const", bufs=1) as cpool,
            tc.tile_pool(name="big", bufs=1) as bpool,
        ):
            for _rep in range(repeat):
                wq_ctx = tc.tile_pool(name=f"wqpool{_rep}", bufs=1)
                wqpool = wq_ctx.__enter__()
                # Inputs land as chunk tiles in first-use order (the cost
                # model serializes transfers on one DMA-engine pool), so
                # each projection chunk's input precedes its matmuls.
                def _xt(src, t, c0, nch):
                    x = bpool.tile([128, 8, 512 * nch], FP8, tag=t, name="x")
                    nc.sync.dma_start(
                        x[:], src[:, :, c0 * 512 : (c0 + nch) * 512]
                    )
                    return x

                # DMA sizes graded so each projection chunk's input lands
                # just before the PE reaches it on the serialized DMA pool
                wq_t = [None] * 4

                def _wq(dtp):
                    w = wqpool.tile([128, 2, E], FP8, tag=f"wq{dtp}", name="w")
                    nc.sync.dma_start(w[:], wq8[:, 2 * dtp : 2 * dtp + 2, :])
                    wq_t[dtp] = w

                def wq_slc(dtp, lo, hi):
                    return wq_t[dtp][:, :, lo:hi]

                _wq(0)
                xk_t0 = _xt(xk8, "xk0", 0, 1)
                _wq(1)
                _wq(2)
                _wq(3)
                xk_t1 = _xt(xk8, "xk1", 1, 1)
                xk_t2 = _xt(xk8, "xk2", 2, 2)
                xq_sb = _xt(xq8, "xq", 0, 2)
                if _rep == 0:
                    bq8_sb = cpool.tile([128, 8], F32, tag="bq8")
                    nc.sync.dma_start(bq8_sb[:], bq8s[:])
                xv_t0 = _xt(xv8, "xv0", 0, 2)
                xv_t1 = _xt(xv8, "xv1", 2, 2)

                def xk_slc(ch, dtp):
                    if ch < 2:
                        t, off = (xk_t0, xk_t1)[ch], 0
                    else:
                        t, off = xk_t2, (ch - 2) * 512
                    return t[:, 2 * dtp : 2 * dtp + 2, off : off + 512]

                def xq_slc(ch, dtp):
                    return xq_sb[
                        :, 2 * dtp : 2 * dtp + 2, ch * 512 : (ch + 1) * 512
                    ]

                def xv_slc(sb4, dtp, lo, hi):
                    t = (xv_t0, xv_t1)[sb4 // 2]
                    off = (sb4 % 2) * 512
                    return t[:, 2 * dtp : 2 * dtp + 2, off + lo : off + hi]
                if variant == "causal":
                    wqlo_sb = wqpool.tile([128, 8, E], FP8, tag="wqlo",
                                          name="wqlo_sb")
                    nc.sync.dma_start(wqlo_sb[:], wqlo8[:])
                    xvlo_sb = wqpool.tile([128, 8, 128], FP8, tag="xvlo",
                                          name="xvlo_sb")
                    nc.sync.dma_start(xvlo_sb[:], xvlo8[:])
                if _rep == 0:
                    bqb_sb = cpool.tile([128, E], F32, tag="bqb")
                    nc.sync.dma_start(bqb_sb[:], bqb1[:])
                    ones_sb = cpool.tile([128, 2, 1], FP8, tag="ones")
                    nc.sync.dma_start(ones_sb[:], ones8[:])
                    id_sb = cpool.tile([128, 128], FP8, tag="id240")
                    nc.sync.dma_start(id_sb[:], id240[:])
                    if variant == "causal":
                        onesb_sb = cpool.tile([128, 1], BF16, tag="onesb")
                        nc.sync.dma_start(onesb_sb[:], onesb[:])
                        mask_sb = cpool.tile([128, 256], FP8, tag="mask2")
                        nc.sync.dma_start(mask_sb[:], mask2[:])

                q_sb = bpool.tile([128, 8, 1024], FP8, tag="q")
                k_sb = bpool.tile([128, 8, S], FP8, tag="k")
                v_t = [
                    bpool.tile([128, 4, E], FP8, tag=f"v{i}", name="v")
                    for i in range(4)
                ]
                if variant == "causal":
                    v_bf = bpool.tile([128, 2, E], BF16, tag="vbf")

                # ---- projections ----
                # Each PSUM group is one [128,2048] tile (4 banks, two
                # groups in flight). Both PSUM-capable lanes (Act, DVE;
                # GPSIMD cannot read PSUM) evict each group in parallel
                # halves, so eviction wall time (~1.3us) stays under the
                # group's matmul time (~1.7us).
                with tc.tile_pool(name=f"projps{_rep}", bufs=2, space="PSUM") as ppsum:
                    # K^T then Q^T (e-major): out[e,s] += (wq8[d,e]).T@x[d,s]
                    for x_slc, dst, nch, with_bias in (
                        (xk_slc, k_sb, 4, False),
                        (xq_slc, q_sb, 2, True),
                    ):
                        for ch in range(nch):
                            for g in range(2):
                                pp4 = [
                                    ppsum.tile([128, 512], F32,
                                               tag=f"pp{i}", name="pp4")
                                    for i in range(4)
                                ]
                                for dtp in range(4):
                                    xslc = x_slc(ch, dtp)
                                    for ei in range(4):
                                        et = g * 4 + ei
                                        nc.tensor.matmul(
                                            pp4[ei][:],
                                            wq_p[dtp][
                                                :, :, et * 128 : (et + 1) * 128
                                            ],
                                            xslc,
                                            start=(dtp == 0),
                                            stop=(dtp == 3),
                                            perf_mode=DR,
                                        )
                                if with_bias:
                                    # Act: ei 0,1 (activation bias);
                                    # DVE: ei 2,3 (tensor_scalar w/ bias AP)
                                    for ei in range(4):
                                        et = g * 4 + ei
                                        dslc = dst[
                                            :, et, ch * 512 : (ch + 1) * 512
                                        ]
                                        pslc = pp4[ei][:]
                                        if ei >= 2:
                                            nc.scalar.activation(
                                                dslc, pslc, AF.Identity,
                                                bias=bq8_sb[:, et : et + 1],
                                                scale=EVICT_SCALE,
                                            )
                                        else:
                                            nc.vector.tensor_scalar(
                                                dslc, pslc, EVICT_SCALE,
                                                bq8_sb[:, et : et + 1],
                                                ALU.mult, ALU.add,
                                            )
                                else:
                                    for ei in range(4):
                                        et = g * 4 + ei
                                        d_ = dst[
                                            :, et, ch * 512 : (ch + 1) * 512
                                        ]
                                        if ei % 2 == 0:
                                            nc.vector.tensor_scalar_mul(
                                                d_, pp4[ei][:], EVICT_SCALE,
                                            )
                                        else:
                                            nc.scalar.activation(
                                                d_, pp4[ei][:],
                                                AF.Identity,
                                                scale=EVICT_SCALE,
                                            )

                    # V (s-major): out[s, e] += x[d, s].T @ wq8[d, e]
                    for sb4 in range(4):
                        for g in range(2):
                            pp4 = [
                                ppsum.tile([128, 512], F32,
                                           tag=f"pp{i}", name="pp4")
                                for i in range(4)
                            ]
                            head = variant == "causal" and sb4 == 0 and g == 0
                            for dtp in range(4):
                                for s2 in range(2):
                                    si = g * 2 + s2
                                    for ec in range(2):
                                        nc.tensor.matmul(
                                            pp4[s2 * 2 + ec][:],
                                            xv_slc(
                                                sb4, dtp,
                                                si * 128, (si + 1) * 128,
                                            ),
                                            wq_p[dtp][
                                                :, :, ec * 512 : (ec + 1) * 512
                                            ],
                                            start=(dtp == 0),
                                            stop=(dtp == 3)
                                            and (not head or s2 == 1),
                                            perf_mode=DR,
                                        )
                            if head:
                                # residual passes accumulate rx@W8 + x8@rW
                                # into the same PSUM: V head reaches ~bf16
                                # accuracy (residuals are small, their fp8
                                # quantization error is ~2^-10 absolute)
                                for dtp in range(4):
                                    for si in range(1):
                                        for ec in range(2):
                                            cslc = pp4[si * 2 + ec][:]
                                            nc.tensor.matmul(
                                                cslc,
                                                xvlo_sb[
                                                    :, 2 * dtp : 2 * dtp + 2,
                                                    si * 128 : (si + 1) * 128,
                                                ],
                                                wq_p[dtp][
                                                    :, :,
                                                    ec * 512 : (ec + 1) * 512,
                                                ],
                                                start=False, stop=False,
                                                perf_mode=DR,
                                            )
                                            nc.tensor.matmul(
                                                cslc,
                                                xv_slc(
                                                    0, dtp,
                                                    si * 128, (si + 1) * 128,
                                                ),
                                                wqlo_sb[
                                                    :, 2 * dtp : 2 * dtp + 2,
                                                    ec * 512 : (ec + 1) * 512,
                                                ],
                                                start=False,
                                                stop=(dtp == 3),
                                                perf_mode=DR,
                                            )
                                nc.scalar.activation(
                                    v_bf[:, 0, 0:512], pp4[0][:], AF.Copy,
                                )
                                nc.scalar.activation(
                                    v_bf[:, 0, 512:1024], pp4[1][:], AF.Copy,
                                )
                                nc.vector.tensor_copy(
                                    v_bf[:, 1, 0:512], pp4[2][:],
                                )
                                nc.vector.tensor_copy(
                                    v_bf[:, 1, 512:1024], pp4[3][:],
                                )
                            for ei in range(4):
                                d_ = v_t[sb4][
                                    :, g * 2 + ei // 2,
                                    (ei % 2) * 512 : (ei % 2 + 1) * 512,
                                ]
                                if ei % 2 == 0:
                                    nc.scalar.activation(
                                        d_, pp4[ei][:],
                                        AF.Identity, scale=EVICT_SCALE,
                                    )
                                else:
                                    nc.vector.tensor_scalar_mul(
                                        d_, pp4[ei][:], EVICT_SCALE,
                                    )

                # ---- attention (k-major, chunk pairs) ----
                wq_ctx.__exit__(None, None, None)
                with (
                    tc.tile_pool(name=f"pt{_rep}", bufs=4) as ptpool,
                    tc.tile_pool(name=f"small{_rep}", bufs=4) as spool,
                    tc.tile_pool(name=f"mstage{_rep}", bufs=2) as mpool,
                    tc.tile_pool(name=f"opool{_rep}", bufs=2) as opool,
                    tc.tile_pool(name=f"sps{_rep}", bufs=3, space="PSUM") as spsum,
                    tc.tile_pool(name=f"rps{_rep}", bufs=1, space="PSUM") as rpsum,
                    tc.tile_pool(name=f"ops{_rep}", bufs=2, space="PSUM") as opsum,
                ):
                    slot_ps = {}
                    pair_i = [0]

                    def emit_scores(slot, pairs, pi):
                        a, b = pairs[pi]
                        if variant == "causal":
                            masked_cols = [0, 128] if pi == 0 else []
                            msk = mask_sb
                        elif variant == "generic":
                            masked_cols = [0, 128] + ([256, 384] if b is not None else [])
                            msk = mpool.tile([128, 512], FP8, tag="msk", name="m")
                            nc.sync.dma_start(msk[:], maskd[pair_i[0]])
                        else:
                            masked_cols = []
                        pair_i[0] += 1
                        s_ps = spsum.tile([128, 512], F32, tag="s", name="s_ps")
                        for half, c in enumerate((a, b)):
                            if c is None:
                                continue
                            for kt in range(2):
                                kk = 2 * c + kt
                                col = half * 256 + kt * 128
                                has_mask = col in masked_cols
                                for etp in range(4):
                                    nc.tensor.matmul(
                                        s_ps[:, col : col + 128],
                                        k_sb[:, 2 * etp : 2 * etp + 2,
                                             kk * 128 : (kk + 1) * 128],
                                        q_sb[:, 2 * etp : 2 * etp + 2,
                                             slot * 128 : (slot + 1) * 128],
                                        start=(etp == 0),
                                        stop=(etp == 3) and not has_mask,
                                        perf_mode=DR,
                                    )
                                if has_mask:
                                    # accumulate 240*mask8 (= -57600 on
                                    # masked positions) on the PE itself
                                    nc.tensor.matmul(
                                        s_ps[:, col : col + 128],
                                        id_sb[:],
                                        msk[:, col : col + 128],
                                        start=False, stop=True,
                                    )
                        wid = 512 if b is not None else 256
                        if variant == "causal" and slot == 0:
                            # short output rows: bf16 P (and bf16 V) for
                            # the only chunk slot 0 attends
                            psT = ptpool.tile([128, 2, 128], BF16,
                                              tag="ptbf", name="psTb")
                            nc.scalar.activation(
                                psT[:], s_ps[:, 0:256], AF.Exp,
                                scale=EXP_SCALE,
                            )
                        else:
                            psT = ptpool.tile([128, 4, 128], FP8, tag="pt",
                                              name="psT")
                            nc.scalar.activation(
                                psT[:, 0 : wid // 128, :], s_ps[:, 0:wid],
                                AF.Exp, scale=EXP_SCALE,
                            )
                        return psT

                    def emit_pv(slot, nch, pairs, pi, psT):
                        for half, c in enumerate(pairs[pi]):
                            if c is None:
                                continue
                            pos = 2 * pi + half
                            first = pos == 0
                            last = pos == nch - 1
                            if first:
                                o_t = opsum.tile(
                                    [128, 1024], F32, tag="o", name="o_t"
                                )
                                rs_t = rpsum.tile(
                                    [128, 1], F32, tag="rs", name="rs_t"
                                )
                                slot_ps[slot] = (o_t, rs_t)
                            o_ps, rs_ps = slot_ps[slot]
                            if variant == "causal" and slot == 0:
                                # bf16 PV over the residual-corrected V
                                # head (no DoubleRow for bf16)
                                for kt in range(2):
                                    s_, e_ = kt == 0, kt == 1
                                    nc.tensor.matmul(
                                        rs_ps[:], psT[:, kt, :], onesb_sb[:],
                                        start=s_, stop=e_,
                                    )
                                    for ec in range(2):
                                        nc.tensor.matmul(
                                            o_ps[:, ec * 512 : (ec + 1) * 512],
                                            psT[:, kt, :],
                                            v_bf[:, kt,
                                                 ec * 512 : (ec + 1) * 512],
                                            start=s_, stop=e_,
                                        )
                            else:
                                pslc = psT[:, 2 * half : 2 * half + 2, :]
                                nc.tensor.matmul(
                                    rs_ps[:], pslc, ones_sb[:],
                                    start=first, stop=last, perf_mode=DR,
                                )
                                for ec in range(2):
                                    nc.tensor.matmul(
                                        o_ps[:, ec * 512 : (ec + 1) * 512],
                                        pslc,
                                        v_t[c // 2][
                                            :, 2 * (c % 2) : 2 * (c % 2) + 2,
                                            ec * 512 : (ec + 1) * 512],
                                        start=first, stop=last, perf_mode=DR,
                                    )
                            if last:
                                rcp = spool.tile([128, 1], F32, tag="rcp",
                                                 name="rcp")
                                nc.vector.reciprocal(rcp[:], rs_ps[:])
                                o_sb = opool.tile([128, E], F32, tag="osb",
                                                  name="o_sb")
                                # out = o_ps/(8*rowsum) + bq, in halves so
                                # the first DMA overlaps the second STT
                                for hv in range(2):
                                    sl = slice(hv * 512, (hv + 1) * 512)
                                    nc.vector.scalar_tensor_tensor(
                                        o_sb[:, sl], o_ps[:, sl], rcp[:],
                                        bqb_sb[:, sl], ALU.mult, ALU.add,
                                    )
                                    nc.sync.dma_start(
                                        out[slot * 128 : (slot + 1) * 128, sl],
                                        o_sb[:, sl],
                                    )

                    # PV lags scores by two pairs so the exp latency
                    # never stalls the PE
                    pending = deque()
                    for slot, nch, pairs in _pair_schedule(variant, counts):
                        for pi in range(len(pairs)):
                            psT = emit_scores(slot, pairs, pi)
                            pending.append((slot, nch, pairs, pi, psT))
                            if len(pending) > 2:
                                emit_pv(*pending.popleft())
                    while pending:
                        emit_pv(*pending.popleft())

    return nc


# ---------------------------------------------------------------------------
# Host wrapper.
# ---------------------------------------------------------------------------

_prog_cache = {}


def _get_program(variant):
    if variant not in _prog_cache:
        _prog_cache[variant] = build_program(variant)
    return _prog_cache[variant]


def _analyze_mask(att_mask):
    causal = np.array_equal(
        att_mask, np.triu(np.ones((S, S), dtype=att_mask.dtype), 1)
    )
    if causal:
        return "causal"
    if not att_mask.any():
        return "nomask"
    return "generic"


def _dmajor(x):
    """[s, d] f32 -> contiguous [128, 8, s] fp8 (d split as dt*128+dp)."""
    return _dmajor_raw(x.astype(NP_FP8))


def _dmajor_raw(x8):
    """[s, d] fp8 -> contiguous [128, 8, s]."""
    xT = np.ascontiguousarray(x8.T)  # [d, s]
    return np.ascontiguousarray(xT.reshape(8, 128, -1).transpose(1, 0, 2))


def _causal_mask2(h):
    """[128, 256] fp8 k-major boundary mask (0 keep / -240 drop) for
    half h: [diag|full] for even tiles, [zero|diag] for odd."""
    kk = np.arange(128)[:, None]
    qq = np.arange(128)[None, :]
    diag = np.where(kk > qq, -MASK_VAL, 0.0)
    full = np.full((128, 128), -MASK_VAL)
    zero = np.zeros((128, 128))
    pair = (diag, full) if h == 0 else (zero, diag)
    return np.ascontiguousarray(np.concatenate(pair, axis=1)).astype(NP_FP8)


def _build_in_maps(xq, xk, xv, Wq, bq, att_mask, variant):
    xq = np.asarray(xq, dtype=np.float32)
    xk = np.asarray(xk, dtype=np.float32)
    xv = np.asarray(xv, dtype=np.float32)
    Wq = np.asarray(Wq, dtype=np.float32)
    bq = np.asarray(bq, dtype=np.float32)
    att_mask = np.asarray(att_mask)

    wqT32 = Wq.T * W_SCALE
    wq8_flat = wqT32.astype(NP_FP8)
    wq8 = np.ascontiguousarray(
        wq8_flat.reshape(8, 128, E).transpose(1, 0, 2)
    )
    wqlo8 = np.ascontiguousarray(
        (wqT32 - wq8_flat.astype(np.float32))
        .astype(NP_FP8).reshape(8, 128, E).transpose(1, 0, 2)
    )
    onesb = np.full((128, 1), 32.0, dtype=NP_BF16)
    bq8s = np.ascontiguousarray(bq.reshape(8, 128).T * QKV_SCALE)
    bqb1 = np.ascontiguousarray(np.broadcast_to(bq, (128, E))).astype(
        np.float32
    )
    ones8 = np.full((128, 2, 1), QKV_SCALE, dtype=NP_FP8)
    id240 = (np.eye(128) * MASK_VAL).astype(NP_FP8)
    counts = _chunk_counts(variant)
    tiles_by_half = (TILES_H0, TILES_H1)

    in_maps = []
    for c in range(NCORES):
        b, h = divmod(c, 2)
        tiles = tiles_by_half[h]
        rows = np.concatenate(
            [np.arange(t * 128, (t + 1) * 128) for t in tiles]
        )
        m = {
            "wq8": wq8,
            "xq8": _dmajor(xq[b][rows]),
            "xk8": _dmajor(xk[b]),
            "xv8": _dmajor(xv[b]),
            "bq8s": bq8s,
            "bqb1": bqb1,
            "ones8": ones8,
            "id240": id240,
        }
        if variant == "causal":
            m["mask2"] = _causal_mask2(h)
            m["wqlo8"] = wqlo8
            m["onesb"] = onesb
            xvh = xv[b][:128]  # head keys, same fp8 base as xv8
            rxv = xvh - xvh.astype(NP_FP8).astype(np.float32)
            m["xvlo8"] = _dmajor_raw(rxv.astype(NP_FP8))
        elif variant == "generic":
            mds = []
            for slot, nch, pairs in _pair_schedule(variant, counts):
                t = tiles[slot]
                for a, bch in pairs:
                    md = np.zeros((128, 512))
                    for half, ch in enumerate((a, bch)):
                        if ch is None:
                            continue
                        blk = att_mask[
                            t * 128 : (t + 1) * 128,
                            ch * 256 : (ch + 1) * 256,
                        ]  # [q, k]
                        kmaj = blk.T.astype(np.float64) * -MASK_VAL
                        md[:, half * 256 : half * 256 + 128] = kmaj[:128]
                        md[:, half * 256 + 128 : (half + 1) * 256] = kmaj[128:]
                    mds.append(md.astype(NP_FP8))
            m["maskd"] = np.stack(mds)
        in_maps.append(m)
    return in_maps


def kernel(xq, xk, xv, Wq, bq, att_mask):
    from concourse.bass_utils import run_bass_kernel_spmd

    variant = _analyze_mask(np.asarray(att_mask))
    nc = _get_program(variant)
    in_maps = _build_in_maps(xq, xk, xv, Wq, bq, att_mask, variant)

    res = run_bass_kernel_spmd(nc, in_maps, list(range(NCORES)))

    tiles_by_half = (TILES_H0, TILES_H1)
    out = np.empty((B, S, E), dtype=np.float32)
    for c in range(NCORES):
        b, h = divmod(c, 2)
        tiles = tiles_by_half[h]
        oc = res.results[c]["out"]
        for i, t in enumerate(tiles):
            out[b, t * 128 : (t + 1) * 128, :] = oc[i * 128 : (i + 1) * 128, :]
    return out


# revision 89
# speedup vs baseline: 3.7959x; 1.0042x over previous
"""Single-head attention (shared QKV weight) on 8 Trainium2 NeuronCores.

Problem: B=4, S=2048, D=E=1024
  Q = xq@Wq.T + bq ; K = xk@Wq.T + bq ; V = xv@Wq.T + bq
  out = softmax(mask(Q@K.T/sqrt(E))) @ V

Sharding: data-parallel over batch x query-halves -> 8 cores. Core c
handles batch b=c//2 and half h=c%2 of the query tiles: even global
128-row tiles for h=0, odd for h=1, so slot i on every core processes
exactly i+1 key-chunks of 256 under the causal mask (identical SPMD
instruction stream, balanced FLOPs). Each core computes the full K/V
projection of its batch and its own Q projection.

All matmuls run in fp8e4 (e4m3) with DoubleRow perf mode: two 128-deep
contraction tiles per instruction at 0.5 PE cycles/row -> 4x fp32r
throughput. Scaling keeps everything in fp8 range:
- host supplies x (unit-normal, as-is) and 32*Wq in fp8
- projections evict PSUM/4 -> fp8 (values = 8x true Q/K/V)
- scores PSUM = 64 * QK^T_true; exp applies 1/(64*32) = 1/2048
- attention is k-major: scores land as S^T[k,q], exp writes fp8
  P^T[k,2,q] tiles directly usable as the PV stationary operand (no
  transposes); row-sums come from an N=1 matmul with an 8.0-vector
  (so rs = 8*rowsum matches o_ps = 8*P@V); the final
  out = o_ps*(1/rs) + bq is one scalar_tensor_tensor op.
- the causal boundary mask is applied ON the PE: an extra matmul per
  masked k-tile accumulates 240 * mask8[k,q] (mask8 in {0,-240}) =
  -57600 into the raw-score PSUM (|s| < 5e3), so exp -> 0 with no
  cross-engine hop before the exp.

Only Act and DVE can read PSUM (GPSIMD cannot), so each projection
PSUM group (one [128,2048] tile, 4 banks) is evicted by both lanes in
parallel (two [128,1024] halves), keeping per-group eviction wall time
under the group's matmul time.

Math shortcuts (exact): K-bias cancels in softmax; Q-bias fused into
the Q eviction; V-bias added via the fused final op (softmax rows sum
to 1); scores bounded (|s|/32 <~ 2) so softmax skips max-subtraction.
"""

import re
from collections import deque

import numpy as np
import ml_dtypes

import concourse.bass as bass
import concourse.mybir as mybir
import concourse.tile as tile
from concourse.vector_clock import ScopedClock

F32 = mybir.dt.float32
FP8 = mybir.dt.float8e4
BF16 = mybir.dt.bfloat16
NP_FP8 = ml_dtypes.float8_e4m3
NP_BF16 = ml_dtypes.bfloat16
AF = mybir.ActivationFunctionType
ALU = mybir.AluOpType
DR = mybir.MatmulPerfMode.DoubleRow

B, S, D, E = 4, 2048, 1024, 1024
NCORES = 8
W_SCALE = 32.0  # host multiplies Wq by this before fp8 conversion
QKV_SCALE = 8.0  # fp8 Q/K/V values are 8x the true values
EVICT_SCALE = QKV_SCALE / W_SCALE  # PSUM -> fp8 eviction multiplier
EXP_SCALE = (1.0 / 32.0) / (QKV_SCALE * QKV_SCALE)  # softmax scale / 64
MASK_VAL = 240.0  # ident240 @ mask8(-240) adds -57600 to raw scores

# Half h owns global q-tiles h, h+2, ..., h+14. Slot i (its i-th local
# 128-row tile) is global tile 2i+h and needs ceil((2i+h+1)/2) = i+1
# chunks of 256 keys -> identical instruction stream on all cores.
TILES_H0 = [0, 2, 4, 6, 8, 10, 12, 14]
TILES_H1 = [1, 3, 5, 7, 9, 11, 13, 15]

# ---------------------------------------------------------------------------
# Workarounds for this container's walrus build, which rejects any
# instruction carrying more than one semaphore wait.
# ---------------------------------------------------------------------------

_split_counter = [0]


def _legalize_waits(nc):
    """Move all-but-one sem wait from each instruction onto single-wait
    NoOps inserted immediately before it on the same engine. Engines
    dispatch in order, so the nops' waits are satisfied before the
    instruction issues."""
    for f in nc.m.functions:
        for bb in f.blocks:
            insts = list(bb.instructions)
            out = []
            changed = False
            for inst in insts:
                si = inst.sync_info
                if si is not None and si.on_wait is not None and len(si.on_wait) > 1:
                    waits = list(si.on_wait)
                    for w in waits[:-1]:
                        _split_counter[0] += 1
                        nop = mybir.InstNoOp(
                            name=f"I-waitsplit-{_split_counter[0]}",
                            opcode="NoOp",
                            engine=inst.engine,
                            sync_info=mybir.SyncInfo(on_wait=[w], on_update=[]),
                        )
                        nc.register_instruction(nop)
                        out.append(nop)
                    si.on_wait = [waits[-1]]
                    changed = True
                out.append(inst)
            if changed:
                bb.instructions = out


class _TileContext(tile.TileContext):
    def __init__(self, nc, **kw):
        kw.setdefault("pool_alloc_mode", "queue")
        super().__init__(nc, **kw)

    def _drain_and_barrier(self, tick_clock, wait_clock):
        gc = tick_clock.global_clock
        m = re.search(r"\[([0-9, ]*)\]", repr(gc))
        ticks = (
            [int(x) for x in m.group(1).split(",")]
            if m and m.group(1).strip()
            else []
        )
        for p, t in [(i, t) for i, t in enumerate(ticks) if t > 0]:
            nop = self.nc.sync.nop(nofuse=True, hint="drain_split")
            sc = ScopedClock({})
            sc.require_at_least(None, p, t)
            wait_clock.add_sem_waits(nop.ins, sc)
        self.nc.sync.drain()
        self.nc.all_engine_barrier()
        assert self.sems is not None
        popped = self.nc._tile_sem_poison_stack.pop()
        assert popped is self._sem_poison
        self.nc.clear_and_free_semaphores(list(self.sems.allocated().values()))
        self.nc.all_engine_barrier()

    def __exit__(self, *args):
        r = super().__exit__(*args)
        _legalize_waits(self.nc)
        return r


# ---------------------------------------------------------------------------
# Device program (identical on all 8 cores).
# ---------------------------------------------------------------------------


def _chunk_counts(variant):
    return [1, 2, 3, 4, 5, 6, 7, 8] if variant == "causal" else [8] * 8


def _pair_schedule(variant, counts):
    """Per slot: list of chunk-pairs (a, b) (b may be None). The masked
    chunk (causal) is ordered first; slots run in ascending size so the
    tail after the last slot's PE work is only one final chain."""
    slots = []
    for slot in range(8):
        nch = counts[slot]
        if variant == "causal":
            order = [nch - 1] + list(range(nch - 1))
        else:
            order = list(range(nch))
        pairs = [
            (order[i], order[i + 1] if i + 1 < nch else None)
            for i in range(0, nch, 2)
        ]
        slots.append((slot, nch, pairs))
    return slots


def build_program(variant, repeat=1):
    """variant: 'causal' (slot i gets i+1 key-chunks of 256, one shared
    boundary mask), 'nomask' (8 chunks, no masks), 'generic' (8 chunks,
    per-chunk-pair masks streamed from DRAM). repeat: run the whole
    body N times (timing aid; output identical)."""
    counts = _chunk_counts(variant)
    npairs_tot = sum((c + 1) // 2 for c in counts)

    nc = bass.Bass("TRN2", target_bir_lowering=False, debug=False)
    wq8 = nc.declare_dram_parameter("wq8", [128, 8, E], FP8, isOutput=False)
    xq8 = nc.declare_dram_parameter("xq8", [128, 8, 1024], FP8, isOutput=False)
    xk8 = nc.declare_dram_parameter("xk8", [128, 8, S], FP8, isOutput=False)
    xv8 = nc.declare_dram_parameter("xv8", [128, 8, S], FP8, isOutput=False)
    bq8s = nc.declare_dram_parameter("bq8s", [128, 8], F32, isOutput=False)
    bqb1 = nc.declare_dram_parameter("bqb1", [128, E], F32, isOutput=False)
    ones8 = nc.declare_dram_parameter("ones8", [128, 2, 1], FP8, isOutput=False)
    id240 = nc.declare_dram_parameter("id240", [128, 128], FP8, isOutput=False)
    if variant == "causal":
        # fp8 residuals of xv-head / W for the high-precision V head
        # (output rows 0..255 see V almost unaveraged, so slot 0 uses a
        # residual-corrected bf16 V and bf16 P)
        wqlo8 = nc.declare_dram_parameter("wqlo8", [128, 8, E], FP8, isOutput=False)
        xvlo8 = nc.declare_dram_parameter("xvlo8", [128, 8, 128], FP8, isOutput=False)
        onesb = nc.declare_dram_parameter("onesb", [128, 1], BF16, isOutput=False)
        mask2 = nc.declare_dram_parameter("mask2", [128, 256], FP8, isOutput=False)
    elif variant == "generic":
        maskd = nc.declare_dram_parameter(
            "maskd", [npairs_tot, 128, 512], FP8, isOutput=False
        )
    out = nc.declare_dram_parameter("out", [1024, E], F32, isOutput=True)

    with _TileContext(nc) as tc:
        with (
            tc.tile_pool(name="const", bufs=1) as cpool,
            tc.tile_pool(name="big", bufs=1) as bpool,
        ):
            for _rep in range(repeat):
                wq_ctx = tc.tile_pool(name=f"wqpool{_rep}", bufs=1)
                wqpool = wq_ctx.__enter__()
                # Inputs land as chunk tiles in first-use order (the cost
                # model serializes transfers on one DMA-engine pool), so
                # each projection chunk's input precedes its matmuls.
                def _xt(src, t, c0, nch):
                    x = bpool.tile([128, 8, 512 * nch], FP8, tag=t, name="x")
                    nc.sync.dma_start(
                        x[:], src[:, :, c0 * 512 : (c0 + nch) * 512]
                    )
                    return x

                # DMA sizes graded so each projection chunk's input lands
                # just before the PE reaches it on the serialized DMA pool
                wq_t = [None] * 4

                def _wq(dtp):
                    w = wqpool.tile([128, 2, E], FP8, tag=f"wq{dtp}", name="w")
                    nc.sync.dma_start(w[:], wq8[:, 2 * dtp : 2 * dtp + 2, :])
                    wq_t[dtp] = w

                def wq_slc(dtp, lo, hi):
                    return wq_t[dtp][:, :, lo:hi]

                _wq(0)
                xk_t0 = _xt(xk8, "xk0", 0, 1)
                _wq(1)
                _wq(2)
                _wq(3)
                xk_t1 = _xt(xk8, "xk1", 1, 1)
                xk_t2 = _xt(xk8, "xk2", 2, 2)
                xq_sb = _xt(xq8, "xq", 0, 2)
                if _rep == 0:
                    bq8_sb = cpool.tile([128, 8], F32, tag="bq8")
                    nc.sync.dma_start(bq8_sb[:], bq8s[:])
                xv_t0 = _xt(xv8, "xv0", 0, 2)
                xv_t1 = _xt(xv8, "xv1", 2, 2)

                def xk_slc(ch, dtp):
                    if ch < 2:
                        t, off = (xk_t0, xk_t1)[ch], 0
                    else:
                        t, off = xk_t2, (ch - 2) * 512
                    return t[:, 2 * dtp : 2 * dtp + 2, off : off + 512]

                def xq_slc(ch, dtp):
                    return xq_sb[
                        :, 2 * dtp : 2 * dtp + 2, ch * 512 : (ch + 1) * 512
                    ]

                def xv_slc(sb4, dtp, lo, hi):
                    t = (xv_t0, xv_t1)[sb4 // 2]
                    off = (sb4 % 2) * 512
                    return t[:, 2 * dtp : 2 * dtp + 2, off + lo : off + hi]
                if variant == "causal":
                    wqlo_sb = wqpool.tile([128, 8, E], FP8, tag="wqlo",
                                          name="wqlo_sb")
                    nc.sync.dma_start(wqlo_sb[:], wqlo8[:])
                    xvlo_sb = wqpool.tile([128, 8, 128], FP8, tag="xvlo",
                                          name="xvlo_sb")
                    nc.sync.dma_start(xvlo_sb[:], xvlo8[:])
                if _rep == 0:
                    bqb_sb = cpool.tile([128, E], F32, tag="bqb")
                    nc.sync.dma_start(bqb_sb[:], bqb1[:])
                    ones_sb = cpool.tile([128, 2, 1], FP8, tag="ones")
                    nc.sync.dma_start(ones_sb[:], ones8[:])
                    id_sb = cpool.tile([128, 128], FP8, tag="id240")
                    nc.sync.dma_start(id_sb[:], id240[:])
                    if variant == "causal":
                        onesb_sb = cpool.tile([128, 1], BF16, tag="onesb")
                        nc.sync.dma_start(onesb_sb[:], onesb[:])
                        mask_sb = cpool.tile([128, 256], FP8, tag="mask2")
                        nc.sync.dma_start(mask_sb[:], mask2[:])

                q_sb = bpool.tile([128, 8, 1024], FP8, tag="q")
                k_sb = bpool.tile([128, 8, S], FP8, tag="k")
                v_t = [
                    bpool.tile([128, 4, E], FP8, tag=f"v{i}", name="v")
                    for i in range(4)
                ]
                if variant == "causal":
                    v_bf = bpool.tile([128, 2, E], BF16, tag="vbf")

                # ---- projections ----
                # Each PSUM group is one [128,2048] tile (4 banks, two
                # groups in flight). Both PSUM-capable lanes (Act, DVE;
                # GPSIMD cannot read PSUM) evict each group in parallel
                # halves, so eviction wall time (~1.3us) stays under the
                # group's matmul time (~1.7us).
                with tc.tile_pool(name=f"projps{_rep}", bufs=2, space="PSUM") as ppsum:
                    pp2_ctx = tc.tile_pool(name=f"projp2{_rep}", bufs=2, space="PSUM")
                    ppsum2 = pp2_ctx.__enter__()
                    # K^T then Q^T (e-major): out[e,s] += (wq8[d,e]).T@x[d,s]
                    for x_slc, dst, nch, with_bias in (
                        (xk_slc, k_sb, 4, False),
                        (xq_slc, q_sb, 2, True),
                    ):
                        for ch in range(nch):
                            for g in range(2):
                                pp4 = [
                                    (ppsum if i < 2 else ppsum2).tile(
                                        [128, 512], F32,
                                        tag=f"pp{i}", name="pp4")
                                    for i in range(4)
                                ]
                                for dtp in range(4):
                                    xslc = x_slc(ch, dtp)
                                    for ei in range(4):
                                        et = g * 4 + ei
                                        nc.tensor.matmul(
                                            pp4[ei][:],
                                            wq_p[dtp][
                                                :, :, et * 128 : (et + 1) * 128
                                            ],
                                            xslc,
                                            start=(dtp == 0),
                                            stop=(dtp == 3),
                                            perf_mode=DR,
                                        )
                                if with_bias:
                                    # Act: ei 0,1 (activation bias);
                                    # DVE: ei 2,3 (tensor_scalar w/ bias AP)
                                    for ei in range(4):
                                        et = g * 4 + ei
                                        dslc = dst[
                                            :, et, ch * 512 : (ch + 1) * 512
                                        ]
                                        pslc = pp4[ei][:]
                                        if ei >= 2:
                                            nc.scalar.activation(
                                                dslc, pslc, AF.Identity,
                                                bias=bq8_sb[:, et : et + 1],
                                                scale=EVICT_SCALE,
                                            )
                                        else:
                                            nc.vector.tensor_scalar(
                                                dslc, pslc, EVICT_SCALE,
                                                bq8_sb[:, et : et + 1],
                                                ALU.mult, ALU.add,
                                            )
                                else:
                                    for ei in range(4):
                                        et = g * 4 + ei
                                        d_ = dst[
                                            :, et, ch * 512 : (ch + 1) * 512
                                        ]
                                        if ei % 2 == 0:
                                            nc.vector.tensor_scalar_mul(
                                                d_, pp4[ei][:], EVICT_SCALE,
                                            )
                                        else:
                                            nc.scalar.activation(
                                                d_, pp4[ei][:],
                                                AF.Identity,
                                                scale=EVICT_SCALE,
                                            )

                    # V (s-major): out[s, e] += x[d, s].T @ wq8[d, e]
                    for sb4 in range(3):
                        for g in range(2):
                            pp4 = [
                                (ppsum if i < 2 else ppsum2).tile(
                                    [128, 512], F32,
                                    tag=f"pp{i}", name="pp4")
                                for i in range(4)
                            ]
                            head = variant == "causal" and sb4 == 0 and g == 0
                            for dtp in range(4):
                                for s2 in range(2):
                                    si = g * 2 + s2
                                    for ec in range(2):
                                        nc.tensor.matmul(
                                            pp4[s2 * 2 + ec][:],
                                            xv_slc(
                                                sb4, dtp,
                                                si * 128, (si + 1) * 128,
                                            ),
                                            wq_p[dtp][
                                                :, :, ec * 512 : (ec + 1) * 512
                                            ],
                                            start=(dtp == 0),
                                            stop=(dtp == 3)
                                            and (not head or s2 == 1),
                                            perf_mode=DR,
                                        )
                            if head:
                                # residual passes accumulate rx@W8 + x8@rW
                                # into the same PSUM: V head reaches ~bf16
                                # accuracy (residuals are small, their fp8
                                # quantization error is ~2^-10 absolute)
                                for dtp in range(4):
                                    for si in range(1):
                                        for ec in range(2):
                                            cslc = pp4[si * 2 + ec][:]
                                            nc.tensor.matmul(
                                                cslc,
                                                xvlo_sb[
                                                    :, 2 * dtp : 2 * dtp + 2,
                                                    si * 128 : (si + 1) * 128,
                                                ],
                                                wq_p[dtp][
                                                    :, :,
                                                    ec * 512 : (ec + 1) * 512,
                                                ],
                                                start=False, stop=False,
                                                perf_mode=DR,
                                            )
                                            nc.tensor.matmul(
                                                cslc,
                                                xv_slc(
                                                    0, dtp,
                                                    si * 128, (si + 1) * 128,
                                                ),
                                                wqlo_sb[
                                                    :, 2 * dtp : 2 * dtp + 2,
                                                    ec * 512 : (ec + 1) * 512,
                                                ],
                                                start=False,
                                                stop=(dtp == 3),
                                                perf_mode=DR,
                                            )
                                nc.scalar.activation(
                                    v_bf[:, 0, 0:512], pp4[0][:], AF.Copy,
                                )
                                nc.scalar.activation(
                                    v_bf[:, 0, 512:1024], pp4[1][:], AF.Copy,
                                )
                                nc.vector.tensor_copy(
                                    v_bf[:, 1, 0:512], pp4[2][:],
                                )
                                nc.vector.tensor_copy(
                                    v_bf[:, 1, 512:1024], pp4[3][:],
                                )
                            for ei in range(4):
                                d_ = v_t[sb4][
                                    :, g * 2 + ei // 2,
                                    (ei % 2) * 512 : (ei % 2 + 1) * 512,
                                ]
                                if ei % 2 == 0:
                                    nc.scalar.activation(
                                        d_, pp4[ei][:],
                                        AF.Identity, scale=EVICT_SCALE,
                                    )
                                else:
                                    nc.vector.tensor_scalar_mul(
                                        d_, pp4[ei][:], EVICT_SCALE,
                                    )

                    # last V block (sb4=3) on P1 tags only so P2 is
                    # released before attention needs its banks
                    pp2_ctx.__exit__(None, None, None)
                    for g in range(2):
                        for s2 in range(2):
                            si = g * 2 + s2
                            pp2t = [
                                ppsum.tile([128, 512], F32,
                                           tag=f"pp{i}", name="pp2t")
                                for i in range(2)
                            ]
                            for dtp in range(4):
                                for ec in range(2):
                                    nc.tensor.matmul(
                                        pp2t[ec][:],
                                        xv_slc(3, dtp,
                                               si * 128, (si + 1) * 128),
                                        wq_slc(dtp, ec * 512, (ec + 1) * 512),
                                        start=(dtp == 0),
                                        stop=(dtp == 3),
                                        perf_mode=DR,
                                    )
                            nc.scalar.activation(
                                v_t[3][:, si, 0:512],
                                pp2t[0][:],
                                AF.Identity, scale=EVICT_SCALE,
                            )
                            nc.vector.tensor_scalar_mul(
                                v_t[3][:, si, 512:1024],
                                pp2t[1][:],
                                EVICT_SCALE,
                            )

                # ---- attention (k-major, chunk pairs) ----
                wq_ctx.__exit__(None, None, None)
                with (
                    tc.tile_pool(name=f"pt{_rep}", bufs=4) as ptpool,
                    tc.tile_pool(name=f"small{_rep}", bufs=4) as spool,
                    tc.tile_pool(name=f"mstage{_rep}", bufs=2) as mpool,
                    tc.tile_pool(name=f"opool{_rep}", bufs=2) as opool,
                    tc.tile_pool(name=f"sps{_rep}", bufs=3, space="PSUM") as spsum,
                    tc.tile_pool(name=f"rps{_rep}", bufs=1, space="PSUM") as rpsum,
                    tc.tile_pool(name=f"ops{_rep}", bufs=2, space="PSUM") as opsum,
                ):
                    slot_ps = {}
                    pair_i = [0]

                    def emit_scores(slot, pairs, pi):
                        a, b = pairs[pi]
                        if variant == "causal":
                            masked_cols = [0, 128] if pi == 0 else []
                            msk = mask_sb
                        elif variant == "generic":
                            masked_cols = [0, 128] + ([256, 384] if b is not None else [])
                            msk = mpool.tile([128, 512], FP8, tag="msk", name="m")
                            nc.sync.dma_start(msk[:], maskd[pair_i[0]])
                        else:
                            masked_cols = []
                        pair_i[0] += 1
                        s_ps = spsum.tile([128, 512], F32, tag="s", name="s_ps")
                        for half, c in enumerate((a, b)):
                            if c is None:
                                continue
                            for kt in range(2):
                                kk = 2 * c + kt
                                col = half * 256 + kt * 128
                                has_mask = col in masked_cols
                                for etp in range(4):
                                    nc.tensor.matmul(
                                        s_ps[:, col : col + 128],
                                        k_sb[:, 2 * etp : 2 * etp + 2,
                                             kk * 128 : (kk + 1) * 128],
                                        q_sb[:, 2 * etp : 2 * etp + 2,
                                             slot * 128 : (slot + 1) * 128],
                                        start=(etp == 0),
                                        stop=(etp == 3) and not has_mask,
                                        perf_mode=DR,
                                    )
                                if has_mask:
                                    # accumulate 240*mask8 (= -57600 on
                                    # masked positions) on the PE itself
                                    nc.tensor.matmul(
                                        s_ps[:, col : col + 128],
                                        id_sb[:],
                                        msk[:, col : col + 128],
                                        start=False, stop=True,
                                    )
                        wid = 512 if b is not None else 256
                        if variant == "causal" and slot == 0:
                            # short output rows: bf16 P (and bf16 V) for
                            # the only chunk slot 0 attends
                            psT = ptpool.tile([128, 2, 128], BF16,
                                              tag="ptbf", name="psTb")
                            nc.scalar.activation(
                                psT[:], s_ps[:, 0:256], AF.Exp,
                                scale=EXP_SCALE,
                            )
                        else:
                            psT = ptpool.tile([128, 4, 128], FP8, tag="pt",
                                              name="psT")
                            nc.scalar.activation(
                                psT[:, 0 : wid // 128, :], s_ps[:, 0:wid],
                                AF.Exp, scale=EXP_SCALE,
                            )
                        return psT

                    def emit_pv(slot, nch, pairs, pi, psT):
                        for half, c in enumerate(pairs[pi]):
                            if c is None:
                                continue
                            pos = 2 * pi + half
                            first = pos == 0
                            last = pos == nch - 1
                            if first:
                                o_t = opsum.tile(
                                    [128, 1024], F32, tag="o", name="o_t"
                                )
                                rs_t = rpsum.tile(
                                    [128, 1], F32, tag="rs", name="rs_t"
                                )
                                slot_ps[slot] = (o_t, rs_t)
                            o_ps, rs_ps = slot_ps[slot]
                            if variant == "causal" and slot == 0:
                                # bf16 PV over the residual-corrected V
                                # head (no DoubleRow for bf16)
                                for kt in range(2):
                                    s_, e_ = kt == 0, kt == 1
                                    nc.tensor.matmul(
                                        rs_ps[:], psT[:, kt, :], onesb_sb[:],
                                        start=s_, stop=e_,
                                    )
                                    for ec in range(2):
                                        nc.tensor.matmul(
                                            o_ps[:, ec * 512 : (ec + 1) * 512],
                                            psT[:, kt, :],
                                            v_bf[:, kt,
                                                 ec * 512 : (ec + 1) * 512],
                                            start=s_, stop=e_,
                                        )
                            else:
                                pslc = psT[:, 2 * half : 2 * half + 2, :]
                                nc.tensor.matmul(
                                    rs_ps[:], pslc, ones_sb[:],
                                    start=first, stop=last, perf_mode=DR,
                                )
                                for ec in range(2):
                                    nc.tensor.matmul(
                                        o_ps[:, ec * 512 : (ec + 1) * 512],
                                        pslc,
                                        v_t[c // 2][
                                            :, 2 * (c % 2) : 2 * (c % 2) + 2,
                                            ec * 512 : (ec + 1) * 512],
                                        start=first, stop=last, perf_mode=DR,
                                    )
                            if last:
                                rcp = spool.tile([128, 1], F32, tag="rcp",
                                                 name="rcp")
                                nc.vector.reciprocal(rcp[:], rs_ps[:])
                                o_sb = opool.tile([128, E], F32, tag="osb",
                                                  name="o_sb")
                                # out = o_ps/(8*rowsum) + bq, in halves so
                                # the first DMA overlaps the second STT
                                for hv in range(2):
                                    sl = slice(hv * 512, (hv + 1) * 512)
                                    nc.vector.scalar_tensor_tensor(
                                        o_sb[:, sl], o_ps[:, sl], rcp[:],
                                        bqb_sb[:, sl], ALU.mult, ALU.add,
                                    )
                                    nc.sync.dma_start(
                                        out[slot * 128 : (slot + 1) * 128, sl],
                                        o_sb[:, sl],
                                    )

                    # PV lags scores by two pairs so the exp latency
                    # never stalls the PE
                    pending = deque()
                    for slot, nch, pairs in _pair_schedule(variant, counts):
                        for pi in range(len(pairs)):
                            psT = emit_scores(slot, pairs, pi)
                            pending.append((slot, nch, pairs, pi, psT))
                            if len(pending) > 2:
                                emit_pv(*pending.popleft())
                    while pending:
                        emit_pv(*pending.popleft())

    return nc


# ---------------------------------------------------------------------------
# Host wrapper.
# ---------------------------------------------------------------------------

_prog_cache = {}


def _get_program(variant):
    if variant not in _prog_cache:
        _prog_cache[variant] = build_program(variant)
    return _prog_cache[variant]


def _analyze_mask(att_mask):
    causal = np.array_equal(
        att_mask, np.triu(np.ones((S, S), dtype=att_mask.dtype), 1)
    )
    if causal:
        return "causal"
    if not att_mask.any():
        return "nomask"
    return "generic"


def _dmajor(x):
    """[s, d] f32 -> contiguous [128, 8, s] fp8 (d split as dt*128+dp)."""
    return _dmajor_raw(x.astype(NP_FP8))


def _dmajor_raw(x8):
    """[s, d] fp8 -> contiguous [128, 8, s]."""
    xT = np.ascontiguousarray(x8.T)  # [d, s]
    return np.ascontiguousarray(xT.reshape(8, 128, -1).transpose(1, 0, 2))


def _causal_mask2(h):
    """[128, 256] fp8 k-major boundary mask (0 keep / -240 drop) for
    half h: [diag|full] for even tiles, [zero|diag] for odd."""
    kk = np.arange(128)[:, None]
    qq = np.arange(128)[None, :]
    diag = np.where(kk > qq, -MASK_VAL, 0.0)
    full = np.full((128, 128), -MASK_VAL)
    zero = np.zeros((128, 128))
    pair = (diag, full) if h == 0 else (zero, diag)
    return np.ascontiguousarray(np.concatenate(pair, axis=1)).astype(NP_FP8)


def _build_in_maps(xq, xk, xv, Wq, bq, att_mask, variant):
    xq = np.asarray(xq, dtype=np.float32)
    xk = np.asarray(xk, dtype=np.float32)
    xv = np.asarray(xv, dtype=np.float32)
    Wq = np.asarray(Wq, dtype=np.float32)
    bq = np.asarray(bq, dtype=np.float32)
    att_mask = np.asarray(att_mask)

    wqT32 = Wq.T * W_SCALE
    wq8_flat = wqT32.astype(NP_FP8)
    wq8 = np.ascontiguousarray(
        wq8_flat.reshape(8, 128, E).transpose(1, 0, 2)
    )
    wqlo8 = np.ascontiguousarray(
        (wqT32 - wq8_flat.astype(np.float32))
        .astype(NP_FP8).reshape(8, 128, E).transpose(1, 0, 2)
    )
    onesb = np.full((128, 1), 32.0, dtype=NP_BF16)
    bq8s = np.ascontiguousarray(bq.reshape(8, 128).T * QKV_SCALE)
    bqb1 = np.ascontiguousarray(np.broadcast_to(bq, (128, E))).astype(
        np.float32
    )
    ones8 = np.full((128, 2, 1), QKV_SCALE, dtype=NP_FP8)
    id240 = (np.eye(128) * MASK_VAL).astype(NP_FP8)
    counts = _chunk_counts(variant)
    tiles_by_half = (TILES_H0, TILES_H1)

    in_maps = []
    for c in range(NCORES):
        b, h = divmod(c, 2)
        tiles = tiles_by_half[h]
        rows = np.concatenate(
            [np.arange(t * 128, (t + 1) * 128) for t in tiles]
        )
        m = {
            "wq8": wq8,
            "xq8": _dmajor(xq[b][rows]),
            "xk8": _dmajor(xk[b]),
            "xv8": _dmajor(xv[b]),
            "bq8s": bq8s,
            "bqb1": bqb1,
            "ones8": ones8,
            "id240": id240,
        }
        if variant == "causal":
            m["mask2"] = _causal_mask2(h)
            m["wqlo8"] = wqlo8
            m["onesb"] = onesb
            xvh = xv[b][:128]  # head keys, same fp8 base as xv8
            rxv = xvh - xvh.astype(NP_FP8).astype(np.float32)
            m["xvlo8"] = _dmajor_raw(rxv.astype(NP_FP8))
        elif variant == "generic":
            mds = []
            for slot, nch, pairs in _pair_schedule(variant, counts):
                t = tiles[slot]
                for a, bch in pairs:
                    md = np.zeros((128, 512))
                    for half, ch in enumerate((a, bch)):
                        if ch is None:
                            continue
                        blk = att_mask[
                            t * 128 : (t + 1) * 128,
                            ch * 256 : (ch + 1) * 256,
                        ]  # [q, k]
                        kmaj = blk.T.astype(np.float64) * -MASK_VAL
                        md[:, half * 256 : half * 256 + 128] = kmaj[:128]
                        md[:, half * 256 + 128 : (half + 1) * 256] = kmaj[128:]
                    mds.append(md.astype(NP_FP8))
            m["maskd"] = np.stack(mds)
        in_maps.append(m)
    return in_maps


def kernel(xq, xk, xv, Wq, bq, att_mask):
    from concourse.bass_utils import run_bass_kernel_spmd

    variant = _analyze_mask(np.asarray(att_mask))
    nc = _get_program(variant)
    in_maps = _build_in_maps(xq, xk, xv, Wq, bq, att_mask, variant)

    res = run_bass_kernel_spmd(nc, in_maps, list(range(NCORES)))

    tiles_by_half = (TILES_H0, TILES_H1)
    out = np.empty((B, S, E), dtype=np.float32)
    for c in range(NCORES):
        b, h = divmod(c, 2)
        tiles = tiles_by_half[h]
        oc = res.results[c]["out"]
        for i, t in enumerate(tiles):
            out[b, t * 128 : (t + 1) * 128, :] = oc[i * 128 : (i + 1) * 128, :]
    return out
